# revision 2
# baseline (speedup 1.0000x reference)
"""AttentiveDensenet Trainium2 Bass kernel.

Data-parallel over batch B=8 across 8 NeuronCores (1 image per core).
Per layer l (of 4):
  - K/Q/V 1x1 convs as bf16 matmuls with x-tiles as the stationary operand,
    producing position-major [pos, (head, dim)] activations directly
    (avoids any transpose for the attention stage). Bias via a K=1
    ones-row matmul accumulated into PSUM.
  - Attention is per-token over the growing key/val list: score products on
    DVE (bf16, 2x mode), d-reduction on DVE, softmax + exact top-k
    (2nd-smallest via min-of-pairwise-max) on DVE/ACT, weighted sum on
    GPSIMD (products) + DVE (accumulate).
  - o is cast to bf16, bounced through DRAM, and transposed to channel-major
    padded layout with the DMA xbar transpose engine.
  - conv3x3 #1 as 9 shifted 1x1 convs accumulated in PSUM (bf16).
  - BatchNorm stats: per-core per-channel sum/sumsq, AllGathered across the
    8 cores (2KB), summed locally. Training-mode BN; the conv bias ob1
    cancels exactly in BN and is dropped.
  - h1 = relu(A*y1 + B) fused on the scalar engine, written bf16 into the
    padded conv2 input.
  - conv3x3 #2 (bf16) + residual x += gamma*(h2 + ob2) fused via
    scalar_tensor_tensor from PSUM.
"""
import numpy as np
import ml_dtypes

import concourse.bacc as bacc
import concourse.mybir as mybir
import concourse.tile as tile
from concourse import bass_utils

L, C, B, H, W = 4, 256, 8, 32, 32
NH, KD = 8, 64
KH = NH * KD          # 512
HW = H * W            # 1024
P = 128
NC = 8                # cores
TOPK = 4
EPS = 1e-7
BN_EPS = 1e-5
PW = W + 2            # 34
PHW = PW * (H + 2)    # 1156

f32 = mybir.dt.float32
bf16 = mybir.dt.bfloat16
AX = mybir.AxisListType
OP = mybir.AluOpType
ACTF = mybir.ActivationFunctionType

_compiled = {}
DBGL = 0


def _build(ncores=NC, dbg=False, no_cc=False, no_gps=False, no_xpose=False, dense_rhs=False, layers=L, stages=99):
    nc = bacc.Bacc(None, target_bir_lowering=False, debug=False, num_devices=ncores)

    # ---- DRAM I/O (per-core shapes) ----
    xin = nc.dram_tensor("xin", [C, HW], f32, kind="ExternalInput").ap()
    wq = nc.dram_tensor("wq", [L, 2, P, KH], bf16, kind="ExternalInput").ap()
    wk = nc.dram_tensor("wk", [L, 2, P, KH], bf16, kind="ExternalInput").ap()
    wv = nc.dram_tensor("wv", [L, 2, P, KH], bf16, kind="ExternalInput").ap()
    bq = nc.dram_tensor("bq", [L, 1, KH], bf16, kind="ExternalInput").ap()
    bk = nc.dram_tensor("bk", [L, 1, KH], bf16, kind="ExternalInput").ap()
    bv = nc.dram_tensor("bv", [L, 1, KH], bf16, kind="ExternalInput").ap()
    w1 = nc.dram_tensor("w1", [L, 9, 4, 2, P, P], bf16, kind="ExternalInput").ap()
    w2 = nc.dram_tensor("w2", [L, 9, 2, 2, P, P], bf16, kind="ExternalInput").ap()
    bngd = nc.dram_tensor("bngd", [L, 2, P, 1], f32, kind="ExternalInput").ap()
    bnbd = nc.dram_tensor("bnbd", [L, 2, P, 1], f32, kind="ExternalInput").ap()
    gob2d = nc.dram_tensor("gob2d", [L, 2, P, 1], f32, kind="ExternalInput").ap()
    gamd = nc.dram_tensor("gamd", [L, P, 1], f32, kind="ExternalInput").ap()
    out = nc.dram_tensor("out", [C, HW], f32, kind="ExternalOutput").ap()
    dbgt = {}
    if dbg:
        for nm, shp in [("d_q", [P, 8 * KH]), ("d_k", [P, 8 * KH]), ("d_v", [P, 8 * KH]),
                        ("d_S", [P, 320]), ("d_attn", [P, 320]), ("d_o", [P, 8 * KH]),
                        ("d_opad0", [P, PHW]), ("d_y1_0", [P, HW]), ("d_gsum", [P, 4]),
                        ("d_A0", [P, 1]), ("d_B0", [P, 1]), ("d_h1p0", [P, PHW]),
                        ("d_x0", [P, HW])]:
            dbgt[nm] = nc.dram_tensor(nm, shp, f32, kind="ExternalOutput").ap()

    with tile.TileContext(nc) as tc:
        with tc.tile_pool(name="main", bufs=1) as mp, \
             tc.tile_pool(name="prodp", bufs=2) as prodp, \
             tc.tile_pool(name="tmpp", bufs=2) as tmpp, \
             tc.tile_pool(name="wkvp", bufs=4) as wkvp, \
             tc.tile_pool(name="wcp", bufs=12) as wcp, \
             tc.tile_pool(name="biasp", bufs=3) as biasp, \
             tc.tile_pool(name="kqvps", bufs=4, space="PSUM") as kqvps, \
             tc.tile_pool(name="convps", bufs=4, space="PSUM") as convps, \
             tc.tile_pool(name="dramp", bufs=2, space="DRAM") as dramp:

            # persistent tiles
            x = [mp.tile([P, HW], f32, name=f"x{i}") for i in range(2)]
            xb = [mp.tile([P, HW], bf16, name=f"xb{i}") for i in range(2)]
            qbt = mp.tile([P, 8 * KH], bf16, name="qbt")
            kbt = [mp.tile([P, 8 * KH], bf16, name=f"kbt{i}") for i in range(L)]
            vbt = [mp.tile([P, 8 * KH], bf16, name=f"vbt{i}") for i in range(L)]
            S = mp.tile([P, 64 * 5], f32, name="S")
            attn = mp.tile([P, 64 * 5], f32, name="attn")
            attnb = mp.tile([P, 64 * 5], bf16, name="attnb")
            mx = mp.tile([P, 64], f32, name="mx")
            zs = mp.tile([P, 64], f32, name="zs")
            dmin = mp.tile([P, 64], f32, name="dmin")
            mxp = mp.tile([P, 64], f32, name="mxp")
            o = mp.tile([P, 8 * KH], f32, name="o")
            obf = mp.tile([P, 8 * KH], bf16, name="obf")
            opad = [mp.tile([P, PHW + 2], bf16, name=f"opad{i}") for i in range(4)]
            y1 = [mp.tile([P, HW], f32, name=f"y1_{i}") for i in range(2)]
            h1p = [mp.tile([P, PHW + 2], bf16, name=f"h1p{i}") for i in range(2)]
            st = mp.tile([P, 4], f32, name="st")
            gst = mp.tile([P, 32], f32, name="gst")
            gsum = mp.tile([P, 4], f32, name="gsum")
            ones1 = mp.tile([1, P], bf16, name="ones1")
            # per-layer consts (reloaded each layer)
            bngt = [mp.tile([P, 1], f32, name=f"bngt{i}") for i in range(2)]
            bnbt = [mp.tile([P, 1], f32, name=f"bnbt{i}") for i in range(2)]
            gob2t = [mp.tile([P, 1], f32, name=f"gob2t{i}") for i in range(2)]
            gamt = mp.tile([P, 1], f32, name="gamt")
            # BN scratch
            t1 = [mp.tile([P, 1], f32, name=f"t1_{i}") for i in range(2)]
            Ac = [mp.tile([P, 1], f32, name=f"Ac{i}") for i in range(2)]
            Bc = [mp.tile([P, 1], f32, name=f"Bc{i}") for i in range(2)]
            sq = mp.tile([P, 1], f32, name="sq")
            vart = mp.tile([P, 1], f32, name="vart")
            stdt = mp.tile([P, 1], f32, name="stdt")

            # init
            for i in range(2):
                nc.sync.dma_start(x[i][:], xin[i * P:(i + 1) * P, :])
                nc.scalar.copy(xb[i][:], x[i][:])
            for i in range(4):
                nc.vector.memset(opad[i][:], 0)
            for i in range(2):
                nc.vector.memset(h1p[i][:], 0)
            nc.vector.memset(ones1[:], 1.0)
            nc.vector.memset(S[:], 0)
            nc.vector.memset(attn[:], 0)

            S3 = S[:].rearrange("p (g t) -> p g t", t=5)
            at3 = attn[:].rearrange("p (g t) -> p g t", t=5)
            ab3 = attnb[:].rearrange("p (g t) -> p g t", t=5)

            for l in range(layers):
                R = l + 1      # number of real keys
                T = R + 1      # +1 zero key

                # ---- per-layer consts ----
                for i in range(2):
                    nc.sync.dma_start(bngt[i][:], bngd[l, i])
                    nc.sync.dma_start(bnbt[i][:], bnbd[l, i])
                    nc.sync.dma_start(gob2t[i][:], gob2d[l, i])
                nc.sync.dma_start(gamt[:], gamd[l])

                # ---- K/Q/V 1x1 convs, position-major ----
                for name, wdr, bdr, dest in (
                    ("k", wk, bk, kbt[l][:]),
                    ("v", wv, bv, vbt[l][:]),
                    ("q", wq, bq, qbt[:]),
                ):
                    bt = biasp.tile([1, KH], bf16, name=f"bias_{name}_{l}", tag="bias")
                    nc.sync.dma_start(bt[:], bdr[l])
                    wts = []
                    for ct in range(2):
                        wt = wkvp.tile([P, KH], bf16, name=f"w_{name}_{l}_{ct}", tag="wkv")
                        nc.sync.dma_start(wt[:], wdr[l, ct])
                        wts.append(wt)
                    for pb in range(8):
                        ps = kqvps.tile([P, KH], f32, name="kqv_ps")
                        nc.tensor.matmul(ps[:], ones1[:], bt[:], start=True, stop=False)
                        nc.tensor.matmul(ps[:], xb[0][:, pb * P:(pb + 1) * P], wts[0][:],
                                         start=False, stop=False)
                        nc.tensor.matmul(ps[:], xb[1][:, pb * P:(pb + 1) * P], wts[1][:],
                                         start=False, stop=True)
                        nc.scalar.copy(dest[:, pb * KH:(pb + 1) * KH], ps[:])

                # ---- scores ----
                if stages < 2: continue
                for t in range(R):
                    pr = prodp.tile([P, 8 * KH], bf16, name="prodb")
                    nc.vector.tensor_mul(pr[:], qbt[:], kbt[t][:])
                    nc.vector.tensor_reduce(
                        out=S3[:, :, t], in_=pr[:].rearrange("p (g d) -> p g d", d=KD),
                        axis=AX.X, op=OP.add)
                nc.vector.memset(S3[:, :, R:R + 1], 0)  # zero key

                # ---- softmax over T slots ----
                if stages < 3: continue
                nc.vector.tensor_reduce(out=mx[:], in_=S3[:, :, 0:T], axis=AX.X, op=OP.max)
                nc.vector.tensor_tensor(
                    at3[:, :, 0:T], S3[:, :, 0:T],
                    mx[:].unsqueeze(2).broadcast_to([P, 64, T]), OP.subtract)
                nc.scalar.activation(at3[:, :, 0:T], at3[:, :, 0:T], ACTF.Exp)
                nc.vector.tensor_reduce(out=zs[:], in_=at3[:, :, 0:T], axis=AX.X, op=OP.add)
                nc.vector.reciprocal(zs[:], zs[:])
                nc.vector.tensor_tensor(
                    at3[:, :, 0:T], at3[:, :, 0:T],
                    zs[:].unsqueeze(2).broadcast_to([P, 64, T]), OP.mult)

                # ---- sparse top-k (only T=5) ----
                if T > TOPK:
                    first = True
                    for i in range(T):
                        for j in range(i + 1, T):
                            dst = dmin if first else mxp
                            nc.vector.tensor_tensor(
                                dst[:], at3[:, :, i], at3[:, :, j],
                                OP.max)
                            if not first:
                                nc.vector.tensor_tensor(dmin[:], dmin[:], mxp[:], OP.min)
                            first = False
                    nc.vector.tensor_scalar_add(dmin[:], dmin[:], EPS)
                    nc.vector.tensor_tensor(
                        at3[:, :, 0:T], at3[:, :, 0:T],
                        dmin[:].unsqueeze(2).broadcast_to([P, 64, T]), OP.subtract)
                    nc.vector.tensor_scalar_max(at3[:, :, 0:T], at3[:, :, 0:T], 0.0)
                    nc.vector.tensor_reduce(out=zs[:], in_=at3[:, :, 0:T], axis=AX.X,
                                            op=OP.add)
                    nc.vector.tensor_scalar_add(zs[:], zs[:], EPS)
                    nc.vector.reciprocal(zs[:], zs[:])
                    nc.vector.tensor_tensor(
                        at3[:, :, 0:T], at3[:, :, 0:T],
                        zs[:].unsqueeze(2).broadcast_to([P, 64, T]), OP.mult)

                nc.vector.tensor_copy(attnb[:], attn[:])

                # ---- weighted sum: o = sum_t attn_t * v_t ----
                if stages < 4: continue
                o3 = o[:].rearrange("p (g d) -> p g d", d=KD)
                for t in range(R):
                    v3 = vbt[t][:].rearrange("p (g d) -> p g d", d=KD)
                    ab = ab3[:, :, t].unsqueeze(2).broadcast_to([P, 64, KD])
                    eng = nc.vector
                    if t == 0:
                        eng.tensor_tensor(o3, v3, ab, OP.mult)
                    else:
                        tm = tmpp.tile([P, 8 * KH], bf16, name="wtmp")
                        tm3 = tm[:].rearrange("p (g d) -> p g d", d=KD)
                        eng.tensor_tensor(tm3, v3, ab, OP.mult)
                        nc.vector.tensor_add(o[:], o[:], tm[:])

                # ---- o -> bf16 -> DRAM -> xbar transpose -> opad ----
                if stages < 5: continue
                nc.scalar.copy(obf[:], o[:])
                odr = dramp.tile([8 * P, KH], bf16, name="odr")
                nc.sync.dma_start(
                    odr[:].rearrange("(b r) h -> r b h", r=P),
                    obf[:].rearrange("p (b h) -> p b h", h=KH))
                for ht in range(4):
                    obt = tmpp.tile([P, HW], bf16, name="obt", tag="obt")
                    if no_xpose:
                        nc.sync.dma_start(obt[:].rearrange('p (a b) -> p a b', b=KH), odr[0:P * 2].rearrange('(p a) h -> p a h', p=P))
                    else:
                        nc.sync.dma_start_transpose(obt[:], odr[:, ht * P:(ht + 1) * P])
                    opv = opad[ht][:, 0:PHW].rearrange("c (i j) -> c i j", j=PW)
                    nc.sync.dma_start(
                        opv[:, 1:H + 1, 1:W + 1],
                        obt[:].rearrange("c (i j) -> c i j", j=W))

                # ---- conv3x3 #1 (bf16): y1 = W1 * opad ----
                if stages < 6: continue
                CHUNKS = [(0, 15), (15, 15), (30, 2)]
                for co in range(2):
                    for (i0, nr) in CHUNKS:
                        ps = convps.tile([P, 512], f32, name="c1ps", tag="cps")
                        nw = PW * nr
                        for tap in range(9):
                            ty, tx = tap // 3, tap % 3
                            for ci in range(4):
                                wt = wcp.tile([P, P], bf16, name="w1t")
                                nc.sync.dma_start(wt[:], w1[l, tap, ci, co])
                                base = PW * (i0 + ty) + tx
                                nc.tensor.matmul(
                                    ps[:, 0:nw], wt[:], opad[ci][:, base:base + nw],
                                    start=(tap == 0 and ci == 0),
                                    stop=(tap == 8 and ci == 3))
                        nc.scalar.copy(
                            y1[co][:, W * i0:W * (i0 + nr)].rearrange(
                                "c (i j) -> c i j", j=W),
                            ps[:, 0:nw].rearrange("c (i j) -> c i j", j=PW)[:, :, 0:W])

                # ---- BN stats + AllGather ----
                if stages < 7: continue
                for co in range(2):
                    nc.vector.tensor_reduce(out=st[:, 2 * co:2 * co + 1], in_=y1[co][:],
                                            axis=AX.X, op=OP.add)
                    nc.scalar.square(o[:, 0:HW], y1[co][:])
                    nc.vector.tensor_reduce(out=st[:, 2 * co + 1:2 * co + 2],
                                            in_=o[:, 0:HW], axis=AX.X, op=OP.add)
                if no_cc:
                    nc.vector.tensor_scalar_mul(gsum[:], st[:], float(ncores))
                else:
                    cci = dramp.tile([1, 512], f32, name="cci")
                    cco = dramp.tile([ncores, 512], f32, name="cco", addr_space="Shared")
                    nc.sync.dma_start(cci[0].rearrange("(p j) -> p j", j=4), st[:])
                    nc.gpsimd.collective_compute(
                        "AllGather", OP.bypass,
                        replica_groups=[list(range(ncores))],
                        ins=[cci.opt()], outs=[cco.opt()])
                    nc.sync.dma_start(
                        gst[:, 0:4 * ncores].rearrange("p (j s) -> p j s", s=ncores),
                        cco[:].rearrange("s (p j) -> p j s", j=4))
                    nc.vector.tensor_reduce(
                        out=gsum[:], in_=gst[:, 0:4 * ncores].rearrange("p (j s) -> p j s", s=ncores),
                        axis=AX.X, op=OP.add)

                # ---- BN coefficients: A = g/sqrt(var+eps), B = b - mean*A ----
                if stages < 8: continue
                NTOT = float(ncores * HW)
                for co in range(2):
                    nc.vector.tensor_scalar_mul(t1[co][:], gsum[:, 2 * co:2 * co + 1],
                                                1.0 / NTOT)
                    nc.vector.tensor_scalar_mul(vart[:], gsum[:, 2 * co + 1:2 * co + 2],
                                                1.0 / NTOT)
                    nc.vector.tensor_mul(sq[:], t1[co][:], t1[co][:])
                    nc.vector.tensor_sub(vart[:], vart[:], sq[:])
                    nc.vector.tensor_scalar_add(vart[:], vart[:], BN_EPS)
                    nc.scalar.activation(stdt[:], vart[:], ACTF.Sqrt)
                    nc.vector.reciprocal(stdt[:], stdt[:])
                    nc.vector.tensor_mul(Ac[co][:], bngt[co][:], stdt[:])
                    nc.vector.tensor_mul(sq[:], t1[co][:], Ac[co][:])
                    nc.vector.tensor_sub(Bc[co][:], bnbt[co][:], sq[:])
                    # h1 = relu(A*y1 + B), strided bf16 into padded conv2 input
                    h1v = h1p[co][:, 0:PHW].rearrange("c (i j) -> c i j", j=PW)
                    nc.scalar.activation(
                        h1v[:, 1:H + 1, 1:W + 1],
                        y1[co][:].rearrange("c (i j) -> c i j", j=W),
                        ACTF.Relu, bias=Bc[co][:], scale=Ac[co][:])

                # ---- conv3x3 #2 (bf16) + residual update ----
                if stages < 9: continue
                for co in range(2):
                    nc.scalar.add(x[co][:], x[co][:], gob2t[co][:])
                    for (i0, nr) in CHUNKS:
                        ps = convps.tile([P, 512], f32, name="c2ps", tag="cps")
                        nw = PW * nr
                        for tap in range(9):
                            ty, tx = tap // 3, tap % 3
                            for ci in range(2):
                                wt = wcp.tile([P, P], bf16, name="w1t")
                                nc.sync.dma_start(wt[:], w2[l, tap, ci, co])
                                base = PW * (i0 + ty) + tx
                                nc.tensor.matmul(
                                    ps[:, 0:nw], wt[:], h1p[ci][:, base:base + nw],
                                    start=(tap == 0 and ci == 0),
                                    stop=(tap == 8 and ci == 1))
                        xslice = x[co][:, W * i0:W * (i0 + nr)]
                        nc.vector.scalar_tensor_tensor(
                            out=xslice.rearrange("c (i j) -> c i j", j=W),
                            in0=ps[:, 0:nw].rearrange("c (i j) -> c i j", j=PW)[:, :, 0:W],
                            scalar=gamt[:],
                            in1=xslice.rearrange("c (i j) -> c i j", j=W),
                            op0=OP.mult, op1=OP.add)
                    if l < layers - 1:
                        nc.scalar.copy(xb[co][:], x[co][:])
                    else:
                        nc.sync.dma_start(out[co * P:(co + 1) * P, :], x[co][:])
                if dbg and l == DBGL:
                    fcvt = mp.tile([P, 8 * KH], f32, name="fcvt")
                    for nm, src_t in [("d_q", qbt), ("d_k", kbt[l]), ("d_v", vbt[l]),
                                      ("d_o", o)]:
                        nc.vector.tensor_copy(fcvt[:], src_t[:])
                        nc.sync.dma_start(dbgt[nm], fcvt[:])
                    nc.vector.tensor_copy(fcvt[:, 0:320], S[:])
                    nc.sync.dma_start(dbgt["d_S"], fcvt[:, 0:320])
                    nc.vector.tensor_copy(fcvt[:, 0:320], attn[:])
                    nc.sync.dma_start(dbgt["d_attn"], fcvt[:, 0:320])
                    nc.vector.tensor_copy(fcvt[:, 0:PHW], opad[0][:, 0:PHW])
                    nc.sync.dma_start(dbgt["d_opad0"], fcvt[:, 0:PHW])
                    nc.sync.dma_start(dbgt["d_y1_0"], y1[0][:])
                    nc.sync.dma_start(dbgt["d_gsum"], gsum[:])
                    nc.sync.dma_start(dbgt["d_A0"], Ac[0][:])
                    nc.sync.dma_start(dbgt["d_B0"], Bc[0][:])
                    nc.vector.tensor_copy(fcvt[:, 0:PHW], h1p[0][:, 0:PHW])
                    nc.sync.dma_start(dbgt["d_h1p0"], fcvt[:, 0:PHW])
                    nc.sync.dma_start(dbgt["d_x0"], x[0][:])

    nc.compile()
    return nc


def _host_prep(inputs):
    bf = ml_dtypes.bfloat16
    kw, kb, qw, qb = inputs["kw"], inputs["kb"], inputs["qw"], inputs["qb"]
    vw, vb = inputs["vw"], inputs["vb"]
    ow1, ow2 = inputs["ow1"], inputs["ow2"]
    gammas, ob2 = inputs["gammas"], inputs["ob2"]

    def packw(wm):  # [L, KH, C] -> [L, 2, 128, KH]
        return np.ascontiguousarray(
            wm.transpose(0, 2, 1).reshape(L, 2, P, KH)).astype(bf)

    d = {}
    d["wq"] = packw(qw / 8.0)
    d["wk"] = packw(kw)
    d["wv"] = packw(vw)
    d["bq"] = np.ascontiguousarray((qb / 8.0).reshape(L, 1, KH)).astype(bf)
    d["bk"] = np.ascontiguousarray(kb.reshape(L, 1, KH)).astype(bf)
    d["bv"] = np.ascontiguousarray(vb.reshape(L, 1, KH)).astype(bf)
    # ow1 [L, 256, 512, 3, 3] -> [L, tap, ci(4), co(2), a(cin128), b(cout128)]
    a1 = ow1.reshape(L, 2, P, 4, P, 3, 3).transpose(0, 5, 6, 3, 1, 4, 2)
    d["w1"] = np.ascontiguousarray(a1.reshape(L, 9, 4, 2, P, P)).astype(bf)
    a2 = ow2.reshape(L, 2, P, 2, P, 3, 3).transpose(0, 5, 6, 3, 1, 4, 2)
    d["w2"] = np.ascontiguousarray(a2.reshape(L, 9, 2, 2, P, P)).astype(bf)
    d["bngd"] = np.ascontiguousarray(
        inputs["bn_g"].reshape(L, 2, P, 1)).astype(np.float32)
    d["bnbd"] = np.ascontiguousarray(
        inputs["bn_b"].reshape(L, 2, P, 1)).astype(np.float32)
    gob2 = gammas[:, None] * ob2
    d["gob2d"] = np.ascontiguousarray(gob2.reshape(L, 2, P, 1)).astype(np.float32)
    d["gamd"] = np.ascontiguousarray(
        np.broadcast_to(gammas[:, None, None], (L, P, 1))).astype(np.float32)
    return d


def _in_maps(inputs):
    shared = _host_prep(inputs)
    x = np.ascontiguousarray(inputs["x"].reshape(B, C, HW)).astype(np.float32)
    in_maps = []
    for c in range(NC):
        m = dict(shared)
        m["xin"] = x[c]
        in_maps.append(m)
    return in_maps


def kernel(**inputs):
    if "nc" not in _compiled:
        _compiled["nc"] = _build()
    nc = _compiled["nc"]
    in_maps = _in_maps(inputs)
    res = bass_utils.run_bass_kernel_spmd(nc, in_maps, core_ids=list(range(NC)))
    outs = np.stack([res.results[c]["out"] for c in range(NC)])
    return outs.reshape(B, C, H, W).astype(np.float32)



# revision 8
# speedup vs baseline: 2.0897x; 2.0897x over previous
"""AttentiveDensenet Trainium2 Bass kernel (v2).

Data-parallel over batch B=8 across 8 NeuronCores (1 image per core).

Key design points (v2, driven by the v1 HW trace):
  - Conv weights are host-packed into one contiguous DRAM block per
    (layer, conv, co-half) and fetched with a single large DMA well ahead
    of use, so conv matmuls stream back-to-back (v1 issued 36 small weight
    DMAs per chunk and the PE starved, degrading each MM to isolated+cold
    timing).
  - Channel order for q/k/v is d-major (col = d*8 + head) so the score
    d-reduction is a contiguous-halving tree of bf16 tensor_tensor adds
    (2x DVE mode) instead of a 1x tensor_reduce, and the attn-weighted
    v-sum multiplies with a step-1-innermost broadcast AP (2x) instead of
    a step-0 broadcast (1x).
  - o is accumulated in bf16 and transposed to channel-major with 32
    PE-transposes straight into the padded conv input (v1 bounced o
    through DRAM + xbar-transpose, ~25-45us/layer of serial DMA).
  - BN stats (sum, sum-sq) are computed per conv1 chunk (hidden under
    conv1's matmuls) via scalar_tensor_tensor accum_out; only the 2KB
    AllGather + coefficient math + h1 remain exposed.
  - Tiny "heartbeat" matmuls are threaded through the attention/BN
    phases (each depending on a fresh DVE result) so the PE's HAM clock
    gate never sees a >3.4us idle window and matmuls stay at 2.4 GHz.
  - conv chunks are (11, 11, 10) rows so every matmul has N>=340 and
    LDWEIGHTS (~107ns) hides under the matmul (~150ns); v1's (15,15,2)
    left a 68-wide chunk that ran LDWEIGHTS-bound.
"""
import numpy as np
import ml_dtypes

import concourse.bacc as bacc
import concourse.mybir as mybir
import concourse.tile as tile
from concourse import bass_utils
from concourse.masks import make_identity

L, C, B, H, W = 4, 256, 8, 32, 32
NH, KD = 8, 64
KH = NH * KD          # 512
HW = H * W            # 1024
P = 128
NC = 8                # cores
TOPK = 4
EPS = 1e-7
BN_EPS = 1e-5
PW = W + 2            # 34
PHW = PW * (H + 2)    # 1156
CHUNKS = [(0, 11), (11, 11), (22, 10)]

f32 = mybir.dt.float32
bf16 = mybir.dt.bfloat16
AX = mybir.AxisListType
OP = mybir.AluOpType
ACTF = mybir.ActivationFunctionType

_compiled = {}


def _build(ncores=NC, layers=L, no_cc=False, dbg=False, dbgl=0):
    nc = bacc.Bacc(None, target_bir_lowering=False, debug=False, num_devices=ncores)

    # ---- DRAM I/O (per-core shapes) ----
    xin = nc.dram_tensor("xin", [C, HW], f32, kind="ExternalInput").ap()
    wkvd = nc.dram_tensor("wkvd", [L, P, 6 * KH], bf16, kind="ExternalInput").ap()
    bkvd = nc.dram_tensor("bkvd", [L, 1, 3 * KH], bf16, kind="ExternalInput").ap()
    w1d = nc.dram_tensor("w1d", [L, 2, P, 36 * P], bf16, kind="ExternalInput").ap()
    w2d = nc.dram_tensor("w2d", [L, 2, P, 18 * P], bf16, kind="ExternalInput").ap()
    cstd = nc.dram_tensor("cstd", [L, P, 8], f32, kind="ExternalInput").ap()
    out = nc.dram_tensor("out", [C, HW], f32, kind="ExternalOutput").ap()
    dbgt = {}
    if dbg:
        for nm, shp in [("d_q", [P, 8 * KH]), ("d_k", [P, 8 * KH]),
                        ("d_v", [P, 8 * KH]), ("d_S", [P, 320]),
                        ("d_attn", [P, 320]), ("d_o", [P, 8 * KH]),
                        ("d_opad0", [P, PHW]), ("d_opad1", [P, PHW]),
                        ("d_y1_0", [P, HW]),
                        ("d_gsum", [P, 4]), ("d_A0", [P, 1]), ("d_B0", [P, 1]),
                        ("d_h1p0", [P, PHW]), ("d_x0", [P, HW])]:
            dbgt[nm] = nc.dram_tensor(nm, shp, f32, kind="ExternalOutput").ap()

    with tile.TileContext(nc) as tc:
        with tc.tile_pool(name="main", bufs=1) as mp, \
             tc.tile_pool(name="prodp", bufs=2) as prodp, \
             tc.tile_pool(name="wp", bufs=1) as wp, \
             tc.tile_pool(name="wkvp", bufs=2) as wkvp, \
             tc.tile_pool(name="cstp", bufs=2) as cstp, \
             tc.tile_pool(name="kqvps", bufs=2, space="PSUM") as kqvps, \
             tc.tile_pool(name="convps", bufs=2, space="PSUM") as convps, \
             tc.tile_pool(name="tpsp", bufs=3, space="PSUM") as tpsp, \
             tc.tile_pool(name="hbp", bufs=1, space="PSUM") as hbp, \
             tc.tile_pool(name="dramp", bufs=2, space="DRAM") as dramp:

            # ---- persistent tiles ----
            x = [mp.tile([P, HW], f32, name=f"x{i}") for i in range(2)]
            xb = [mp.tile([P, HW], bf16, name=f"xb{i}") for i in range(2)]
            qbt = mp.tile([P, 8 * KH], bf16, name="qbt")
            kbt = [mp.tile([P, 8 * KH], bf16, name=f"kbt{i}") for i in range(L)]
            vbt = [mp.tile([P, 8 * KH], bf16, name=f"vbt{i}") for i in range(L)]
            S = mp.tile([P, 64 * 5], f32, name="S")       # [p, t, g] t-major
            attn = mp.tile([P, 64 * 5], f32, name="attn")
            attnb = mp.tile([P, 64 * 5], bf16, name="attnb")
            zs = mp.tile([P, 64], f32, name="zs")
            dmin = mp.tile([P, 64], f32, name="dmin")
            mxp = mp.tile([P, 64], f32, name="mxp")
            pr = mp.tile([P, 8 * KH], bf16, name="pr")    # scores product / wsum tmp
            r1 = mp.tile([P, 2048], bf16, name="r1")
            r2 = mp.tile([P, 1024], bf16, name="r2")
            r3 = mp.tile([P, 512], bf16, name="r3")
            r4 = mp.tile([P, 256], bf16, name="r4")
            r5 = mp.tile([P, 128], bf16, name="r5")
            o = mp.tile([P, 8 * KH], bf16, name="o")
            opad = [mp.tile([P, PHW + 2], bf16, name=f"opad{i}") for i in range(4)]
            y1 = [mp.tile([P, HW], bf16, name=f"y1_{i}") for i in range(2)]
            h1p = [mp.tile([P, PHW + 2], bf16, name=f"h1p{i}") for i in range(2)]
            scr = mp.tile([P, 512], f32, name="scr")      # stats scratch out
            ssum = mp.tile([P, 8], f32, name="ssum")
            ssq = mp.tile([P, 8], f32, name="ssq")
            st = mp.tile([P, 4], f32, name="st")
            gst = mp.tile([P, 32], f32, name="gst")
            gsum = mp.tile([P, 4], f32, name="gsum")
            ones1 = mp.tile([1, P], bf16, name="ones1")
            onesf = mp.tile([1, P], f32, name="onesf")
            ident = mp.tile([P, P], bf16, name="ident")
            # BN coeff scratch
            t1 = [mp.tile([P, 1], f32, name=f"t1_{i}") for i in range(2)]
            Ac = [mp.tile([P, 1], f32, name=f"Ac{i}") for i in range(2)]
            Bc = [mp.tile([P, 1], f32, name=f"Bc{i}") for i in range(2)]
            sq = mp.tile([P, 1], f32, name="sq")
            vart = mp.tile([P, 1], f32, name="vart")
            stdt = mp.tile([P, 1], f32, name="stdt")

            # ---- init ----
            for i in range(2):
                nc.sync.dma_start(x[i][:], xin[i * P:(i + 1) * P, :])
                nc.scalar.copy(xb[i][:], x[i][:])
            for i in range(4):
                nc.vector.memset(opad[i][:], 0)
            for i in range(2):
                nc.vector.memset(h1p[i][:], 0)
            nc.vector.memset(ones1[:], 1.0)
            nc.vector.memset(onesf[:], 1.0)
            nc.vector.memset(S[:], 0)
            nc.vector.memset(attn[:], 0)
            nc.vector.memset(ssum[:], 0)
            nc.vector.memset(ssq[:], 0)
            make_identity(nc, ident[:])

            # views
            pr3 = pr[:].rearrange("p (g i) -> p g i", g=8)     # [p, pb, 512]
            r13 = r1[:].rearrange("p (g i) -> p g i", g=8)
            r23 = r2[:].rearrange("p (g i) -> p g i", g=8)
            r33 = r3[:].rearrange("p (g i) -> p g i", g=8)
            r43 = r4[:].rearrange("p (g i) -> p g i", g=8)
            r53 = r5[:].rearrange("p (g i) -> p g i", g=8)
            St = S[:].rearrange("p (t g) -> p t g", g=64)      # [p, 5, 64]
            at_t = attn[:].rearrange("p (t g) -> p t g", g=64)
            ab_t = attnb[:].rearrange("p (t g) -> p t g", g=64)
            o4 = o[:].rearrange("p (a d h) -> p a d h", a=8, h=8)
            pr4 = pr[:].rearrange("p (a d h) -> p a d h", a=8, h=8)

            hb_ps = hbp.tile([64, 64], f32, name="hb_ps", tag="hb")

            def heartbeat(src_ap):
                # tiny matmul whose rhs depends on fresh DVE output; keeps
                # the PE HAM activity window from going idle.
                lhs = onesf if src_ap.dtype == f32 else ones1
                n = src_ap.shape[-1]
                nc.tensor.matmul(hb_ps[0:64, 0:n], lhs[0:1, 0:64], src_ap,
                                 start=True, stop=True)

            for l in range(layers):
                R = l + 1      # number of real keys
                T = R + 1      # +1 zero key

                # ---- per-layer weight / const loads (one DMA each) ----
                wkvt = wkvp.tile([P, 6 * KH], bf16, name="wkvt", tag="wkv")
                nc.sync.dma_start(wkvt[:], wkvd[l])
                wkv = wkvt[:].rearrange("p (c n) -> p c n", c=2)
                bkv = cstp.tile([1, 3 * KH], bf16, name="bkv", tag="bkv")
                nc.sync.dma_start(bkv[:], bkvd[l])
                cst = cstp.tile([P, 8], f32, name="cst", tag="cst")
                nc.sync.dma_start(cst[:], cstd[l])
                w1s = [wp.tile([P, 36 * P], bf16, name=f"w1s{co}", tag=f"w1s{co}")
                       for co in range(2)]
                for co in range(2):
                    nc.sync.dma_start(w1s[co][:], w1d[l, co])
                w2s = [wp.tile([P, 18 * P], bf16, name=f"w2s{co}", tag=f"w2s{co}")
                       for co in range(2)]
                for co in range(2):
                    nc.sync.dma_start(w2s[co][:], w2d[l, co])

                # ---- K/Q/V 1x1 convs, position-major, d-major channels ----
                for ni, dest in ((0, kbt[l][:]), (1, qbt[:]), (2, vbt[l][:])):
                    for pb in range(8):
                        ps = kqvps.tile([P, KH], f32, name="kqv_ps")
                        nc.tensor.matmul(ps[:], ones1[:],
                                         bkv[:, ni * KH:(ni + 1) * KH],
                                         start=True, stop=False)
                        for ct in range(2):
                            nc.tensor.matmul(
                                ps[:], xb[ct][:, pb * P:(pb + 1) * P],
                                wkv[:, ct, ni * KH:(ni + 1) * KH],
                                start=False, stop=(ct == 1))
                        nc.scalar.copy(dest[:, pb * KH:(pb + 1) * KH], ps[:])

                # ---- scores: S[:, t, :] = sum_d q*k_t  (bf16 tree) ----
                for t in range(R):
                    prt = prodp.tile([P, 8 * KH], bf16, name="prt", tag="prt")
                    prt3 = prt[:].rearrange("p (g i) -> p g i", g=8)
                    nc.vector.tensor_mul(prt[:], qbt[:], kbt[t][:])
                    nc.vector.tensor_tensor(r13, prt3[:, :, 0:256],
                                            prt3[:, :, 256:512], OP.add)
                    nc.vector.tensor_tensor(r23, r13[:, :, 0:128],
                                            r13[:, :, 128:256], OP.add)
                    heartbeat(r2[0:1, 0:64])
                    nc.vector.tensor_tensor(r33, r23[:, :, 0:64],
                                            r23[:, :, 64:128], OP.add)
                    nc.vector.tensor_tensor(r43, r33[:, :, 0:32],
                                            r33[:, :, 32:64], OP.add)
                    nc.vector.tensor_tensor(r53, r43[:, :, 0:16],
                                            r43[:, :, 16:32], OP.add)
                    nc.vector.tensor_tensor(St[:, t, :].rearrange(
                        "p (g i) -> p g i", g=8),
                        r53[:, :, 0:8], r53[:, :, 8:16], OP.add)
                    heartbeat(S[0:1, t * 64:t * 64 + 64])
                nc.vector.memset(St[:, R, :], 0)  # zero-key slot

                # ---- softmax over T slots (scores are small: skip max-sub) ----
                nc.scalar.activation(attn[:, 0:T * 64], S[:, 0:T * 64], ACTF.Exp)
                nc.vector.tensor_reduce(
                    out=zs[:], in_=attn[:, 0:T * 64].rearrange(
                        "p (t g) -> p g t", t=T),
                    axis=AX.X, op=OP.add)
                nc.vector.reciprocal(zs[:], zs[:])
                heartbeat(zs[0:1, 0:64])
                nc.vector.tensor_tensor(
                    at_t[:, 0:T], at_t[:, 0:T],
                    zs[:].unsqueeze(1).broadcast_to([P, T, 64]), OP.mult)

                # ---- sparse top-k (only T=5) ----
                if T > TOPK:
                    first = True
                    for i in range(T):
                        for j in range(i + 1, T):
                            dst = dmin if first else mxp
                            nc.vector.tensor_tensor(
                                dst[:], at_t[:, i], at_t[:, j], OP.max)
                            if not first:
                                nc.vector.tensor_tensor(dmin[:], dmin[:],
                                                        mxp[:], OP.min)
                            first = False
                    heartbeat(dmin[0:1, 0:64])
                    nc.vector.tensor_scalar_add(dmin[:], dmin[:], EPS)
                    nc.vector.tensor_tensor(
                        at_t[:, 0:T], at_t[:, 0:T],
                        dmin[:].unsqueeze(1).broadcast_to([P, T, 64]),
                        OP.subtract)
                    nc.vector.tensor_scalar_max(attn[:, 0:T * 64],
                                                attn[:, 0:T * 64], 0.0)
                    nc.vector.tensor_reduce(
                        out=zs[:], in_=attn[:, 0:T * 64].rearrange(
                            "p (t g) -> p g t", t=T),
                        axis=AX.X, op=OP.add)
                    nc.vector.tensor_scalar_add(zs[:], zs[:], EPS)
                    nc.vector.reciprocal(zs[:], zs[:])
                    heartbeat(zs[0:1, 0:64])
                    nc.vector.tensor_tensor(
                        at_t[:, 0:T], at_t[:, 0:T],
                        zs[:].unsqueeze(1).broadcast_to([P, T, 64]), OP.mult)

                nc.vector.tensor_copy(attnb[:, 0:T * 64], attn[:, 0:T * 64])

                # ---- weighted sum: o = sum_t attn_t * v_t (bf16, 2x APs) ----
                for t in range(R):
                    v4 = vbt[t][:].rearrange("p (a d h) -> p a d h", a=8, h=8)
                    ab4 = ab_t[:, t].rearrange("p (a h) -> p a h", a=8) \
                        .unsqueeze(2).broadcast_to([P, 8, KD, 8])
                    if t == 0:
                        nc.vector.tensor_tensor(o4, v4, ab4, OP.mult)
                    else:
                        nc.vector.tensor_tensor(pr4, v4, ab4, OP.mult)
                        nc.vector.tensor_add(o[:], o[:], pr[:])
                    heartbeat(o[0:1, 0:64])

                # ---- transpose o -> opad (channel-major, padded) ----
                for pb in range(8):
                    for ci in range(4):
                        tps = tpsp.tile([P, P], bf16, name="tps", tag="tps")
                        nc.tensor.transpose(
                            tps[:], o[:, pb * KH + ci * P:pb * KH + ci * P + P],
                            ident[:])
                        opv = opad[ci][:, 0:PHW].rearrange("c (i j) -> c i j",
                                                           j=PW)
                        nc.scalar.copy(
                            opv[:, 1 + 4 * pb:5 + 4 * pb, 1:W + 1],
                            tps[:].rearrange("c (r w) -> c r w", w=W))

                # ---- conv3x3 #1 (bf16): y1 = W1 * opad, stats per chunk ----
                w1v = [w1s[co][:].rearrange("p (t c j) -> p t c j", t=9, c=4)
                       for co in range(2)]
                for co in range(2):
                    for ck, (i0, nr) in enumerate(CHUNKS):
                        ps = convps.tile([P, 512], f32, name="c1ps", tag="cps")
                        nw = PW * nr
                        for tap in range(9):
                            ty, tx = tap // 3, tap % 3
                            base = PW * (i0 + ty) + tx
                            for ci in range(4):
                                nc.tensor.matmul(
                                    ps[:, 0:nw], w1v[co][:, tap, ci],
                                    opad[ci][:, base:base + nw],
                                    start=(tap == 0 and ci == 0),
                                    stop=(tap == 8 and ci == 3))
                        ysl = y1[co][:, W * i0:W * (i0 + nr)]
                        nc.scalar.copy(
                            ysl.rearrange("c (i j) -> c i j", j=W),
                            ps[:, 0:nw].rearrange("c (i j) -> c i j",
                                                  j=PW)[:, :, 0:W])
                        # BN stats for this chunk (hidden under conv matmuls)
                        nc.vector.scalar_tensor_tensor(
                            out=scr[:, 0:W * nr], in0=ysl, scalar=1.0,
                            in1=ysl, op0=OP.mult, op1=OP.mult,
                            accum_out=ssq[:, 4 * co + ck:4 * co + ck + 1])
                        nc.vector.tensor_reduce(
                            out=ssum[:, 4 * co + ck:4 * co + ck + 1],
                            in_=ysl, axis=AX.X, op=OP.add)

                # ---- BN stats total + AllGather ----
                nc.vector.tensor_reduce(
                    out=st[:, 0:2], in_=ssum[:].rearrange(
                        "p (c k) -> p c k", c=2), axis=AX.X, op=OP.add)
                nc.vector.tensor_reduce(
                    out=st[:, 2:4], in_=ssq[:].rearrange(
                        "p (c k) -> p c k", c=2), axis=AX.X, op=OP.add)
                # st layout: [sum_co0, sum_co1, sq_co0, sq_co1]
                if no_cc:
                    nc.vector.tensor_scalar_mul(gsum[:], st[:], float(ncores))
                else:
                    cci = dramp.tile([1, 512], f32, name="cci")
                    cco = dramp.tile([ncores, 512], f32, name="cco",
                                     addr_space="Shared")
                    nc.sync.dma_start(cci[0].rearrange("(p j) -> p j", j=4),
                                      st[:])
                    nc.gpsimd.collective_compute(
                        "AllGather", OP.bypass,
                        replica_groups=[list(range(ncores))],
                        ins=[cci.opt()], outs=[cco.opt()])
                    nc.sync.dma_start(
                        gst[:, 0:4 * ncores].rearrange("p (j s) -> p j s",
                                                       s=ncores),
                        cco[:].rearrange("s (p j) -> p j s", j=4))
                    nc.vector.tensor_reduce(
                        out=gsum[:], in_=gst[:, 0:4 * ncores].rearrange(
                            "p (j s) -> p j s", s=ncores),
                        axis=AX.X, op=OP.add)
                if not no_cc:
                    heartbeat(gst[0:1, 0:32])

                # ---- BN coefficients: A = g/sqrt(var+eps), B = b - mean*A ----
                NTOT = float(ncores * HW)
                for co in range(2):
                    nc.vector.tensor_scalar_mul(t1[co][:],
                                                gsum[:, co:co + 1], 1.0 / NTOT)
                    nc.vector.tensor_scalar_mul(vart[:],
                                                gsum[:, 2 + co:3 + co],
                                                1.0 / NTOT)
                    nc.vector.tensor_mul(sq[:], t1[co][:], t1[co][:])
                    nc.vector.tensor_sub(vart[:], vart[:], sq[:])
                    nc.vector.tensor_scalar_add(vart[:], vart[:], BN_EPS)
                    nc.scalar.activation(stdt[:], vart[:], ACTF.Sqrt)
                    nc.vector.reciprocal(stdt[:], stdt[:])
                    nc.vector.tensor_mul(Ac[co][:], cst[:, co:co + 1], stdt[:])
                    nc.vector.tensor_mul(sq[:], t1[co][:], Ac[co][:])
                    nc.vector.tensor_sub(Bc[co][:], cst[:, 2 + co:3 + co],
                                         sq[:])
                    # h1 = relu(A*y1 + B) into padded conv2 input (bf16)
                    h1v = h1p[co][:, 0:PHW].rearrange("c (i j) -> c i j", j=PW)
                    nc.scalar.activation(
                        h1v[:, 1:H + 1, 1:W + 1],
                        y1[co][:].rearrange("c (i j) -> c i j", j=W),
                        ACTF.Relu, bias=Bc[co][:], scale=Ac[co][:])

                # ---- conv3x3 #2 (bf16) + residual x += gamma*h2 ----
                w2v = [w2s[co][:].rearrange("p (t c j) -> p t c j", t=9, c=2)
                       for co in range(2)]
                for co in range(2):
                    # x += gamma*ob2 (pre-add so stt below fuses residual)
                    nc.scalar.add(x[co][:], x[co][:], cst[:, 4 + co:5 + co])
                    for (i0, nr) in CHUNKS:
                        ps = convps.tile([P, 512], f32, name="c2ps", tag="cps")
                        nw = PW * nr
                        for tap in range(9):
                            ty, tx = tap // 3, tap % 3
                            base = PW * (i0 + ty) + tx
                            for ci in range(2):
                                nc.tensor.matmul(
                                    ps[:, 0:nw], w2v[co][:, tap, ci],
                                    h1p[ci][:, base:base + nw],
                                    start=(tap == 0 and ci == 0),
                                    stop=(tap == 8 and ci == 1))
                        xslice = x[co][:, W * i0:W * (i0 + nr)]
                        nc.vector.scalar_tensor_tensor(
                            out=xslice.rearrange("c (i j) -> c i j", j=W),
                            in0=ps[:, 0:nw].rearrange("c (i j) -> c i j",
                                                      j=PW)[:, :, 0:W],
                            scalar=cst[:, 6:7],
                            in1=xslice.rearrange("c (i j) -> c i j", j=W),
                            op0=OP.mult, op1=OP.add)
                    if l < layers - 1:
                        nc.scalar.copy(xb[co][:], x[co][:])
                    else:
                        nc.sync.dma_start(out[co * P:(co + 1) * P, :], x[co][:])
                if dbg and l == dbgl:
                    def dump(dst, srct, n):
                        for c0 in range(0, n, 2048):
                            cw = min(2048, n - c0)
                            fc = prodp.tile([P, 2048], f32, name="fcvt",
                                            tag="prt")
                            nc.vector.tensor_copy(fc[:, 0:cw],
                                                  srct[:, c0:c0 + cw])
                            nc.sync.dma_start(dst[:, c0:c0 + cw], fc[:, 0:cw])
                    for nm, srct in [("d_q", qbt), ("d_k", kbt[l]),
                                     ("d_v", vbt[l]), ("d_o", o)]:
                        dump(dbgt[nm], srct[:], 8 * KH)
                    nc.sync.dma_start(dbgt["d_S"], S[:])
                    nc.sync.dma_start(dbgt["d_attn"], attn[:])
                    for ci in range(2):
                        dump(dbgt[f"d_opad{ci}"], opad[ci][:], PHW)
                    dump(dbgt["d_y1_0"], y1[0][:], HW)
                    nc.sync.dma_start(dbgt["d_gsum"], gsum[:])
                    nc.sync.dma_start(dbgt["d_A0"], Ac[0][:])
                    nc.sync.dma_start(dbgt["d_B0"], Bc[0][:])
                    dump(dbgt["d_h1p0"], h1p[0][:], PHW)
                    nc.sync.dma_start(dbgt["d_x0"], x[0][:])

    nc.compile()
    return nc


def _host_prep(inputs):
    bf = ml_dtypes.bfloat16
    kw, kb, qw, qb = inputs["kw"], inputs["kb"], inputs["qw"], inputs["qb"]
    vw, vb = inputs["vw"], inputs["vb"]
    ow1, ow2 = inputs["ow1"], inputs["ow2"]
    gammas, ob2 = inputs["gammas"], inputs["ob2"]

    # d-major channel permutation: new col dh -> old col h*64+d
    dh = np.arange(KH)
    perm = (dh % NH) * KD + dh // NH

    def packw(wm):  # [L, KH, C] -> [L, 2, 128, KH] with d-major cols
        return wm.transpose(0, 2, 1)[:, :, perm].reshape(L, 2, P, KH)

    d = {}
    wkv = np.concatenate([packw(kw), packw(qw / 8.0), packw(vw)], axis=3)
    # kernel-side tile is [P, (chunk, col)] -> reorder [L, 2, P, 1536] to
    # [L, P, 2, 1536] before flattening
    d["wkvd"] = np.ascontiguousarray(
        wkv.transpose(0, 2, 1, 3).reshape(L, P, 6 * KH)).astype(bf)
    bkv = np.concatenate([kb[:, perm], (qb / 8.0)[:, perm], vb[:, perm]],
                         axis=1).reshape(L, 1, 3 * KH)
    d["bkvd"] = np.ascontiguousarray(bkv).astype(bf)

    # conv1 weights: [L, co, p(cin in transposed-o order), tap, ci, jo]
    # transposed-o partition p of chtile ci holds original v-channel
    # vh = (p%8)*64 + ci*16 + p//8
    ow1r = ow1.reshape(L, 2, P, KH, 3, 3)  # [l, co, jo, vh, ty, tx]
    w1 = np.empty((L, 2, P, 9, 4, P), np.float32)
    j = np.arange(P)
    for ci in range(4):
        vh = (j % 8) * 64 + ci * 16 + j // 8
        sub = ow1r[:, :, :, vh, :, :]          # [l, co, jo, p, ty, tx]
        w1[:, :, :, :, ci, :] = sub.transpose(0, 1, 3, 4, 5, 2).reshape(
            L, 2, P, 9, P)
    d["w1d"] = np.ascontiguousarray(w1.reshape(L, 2, P, 36 * P)).astype(bf)

    # conv2 weights: [L, co, p(cin), tap, ci, jo]
    a2 = ow2.reshape(L, 2, P, 2, P, 3, 3)      # [l, co, jo, ci, p, ty, tx]
    w2 = a2.transpose(0, 1, 4, 5, 6, 3, 2).reshape(L, 2, P, 9, 2, P)
    d["w2d"] = np.ascontiguousarray(w2.reshape(L, 2, P, 18 * P)).astype(bf)

    # per-layer consts: [bng0, bng1, bnb0, bnb1, gob0, gob1, gam, 0]
    cst = np.zeros((L, P, 8), np.float32)
    bn_g, bn_b = inputs["bn_g"], inputs["bn_b"]
    gob2 = gammas[:, None] * ob2
    for co in range(2):
        cst[:, :, co] = bn_g[:, co * P:(co + 1) * P]
        cst[:, :, 2 + co] = bn_b[:, co * P:(co + 1) * P]
        cst[:, :, 4 + co] = gob2[:, co * P:(co + 1) * P]
    cst[:, :, 6] = gammas[:, None]
    d["cstd"] = np.ascontiguousarray(cst)
    return d


def _in_maps(inputs):
    shared = _host_prep(inputs)
    x = np.ascontiguousarray(inputs["x"].reshape(B, C, HW)).astype(np.float32)
    in_maps = []
    for c in range(NC):
        m = dict(shared)
        m["xin"] = x[c]
        in_maps.append(m)
    return in_maps


def kernel(**inputs):
    if "nc" not in _compiled:
        _compiled["nc"] = _build()
    nc = _compiled["nc"]
    in_maps = _in_maps(inputs)
    res = bass_utils.run_bass_kernel_spmd(nc, in_maps, core_ids=list(range(NC)))
    outs = np.stack([res.results[c]["out"] for c in range(NC)])
    return outs.reshape(B, C, H, W).astype(np.float32)


# revision 9
# speedup vs baseline: 2.4226x; 1.1593x over previous
"""AttentiveDensenet Trainium2 Bass kernel (v2).

Data-parallel over batch B=8 across 8 NeuronCores (1 image per core).

Key design points (v2, driven by the v1 HW trace):
  - Conv weights are host-packed into one contiguous DRAM block per
    (layer, conv, co-half) and fetched with a single large DMA well ahead
    of use, so conv matmuls stream back-to-back (v1 issued 36 small weight
    DMAs per chunk and the PE starved, degrading each MM to isolated+cold
    timing).
  - Channel order for q/k/v is d-major (col = d*8 + head) so the score
    d-reduction is a contiguous-halving tree of bf16 tensor_tensor adds
    (2x DVE mode) instead of a 1x tensor_reduce, and the attn-weighted
    v-sum multiplies with a step-1-innermost broadcast AP (2x) instead of
    a step-0 broadcast (1x).
  - o is accumulated in bf16 and transposed to channel-major with 32
    PE-transposes straight into the padded conv input (v1 bounced o
    through DRAM + xbar-transpose, ~25-45us/layer of serial DMA).
  - BN stats (sum, sum-sq) are computed per conv1 chunk (hidden under
    conv1's matmuls) via scalar_tensor_tensor accum_out; only the 2KB
    AllGather + coefficient math + h1 remain exposed.
  - Tiny "heartbeat" matmuls are threaded through the attention/BN
    phases (each depending on a fresh DVE result) so the PE's HAM clock
    gate never sees a >3.4us idle window and matmuls stay at 2.4 GHz.
  - conv chunks are (11, 11, 10) rows so every matmul has N>=340 and
    LDWEIGHTS (~107ns) hides under the matmul (~150ns); v1's (15,15,2)
    left a 68-wide chunk that ran LDWEIGHTS-bound.
"""
import numpy as np
import ml_dtypes

import concourse.bacc as bacc
import concourse.mybir as mybir
import concourse.tile as tile
from concourse import bass_utils
from concourse.masks import make_identity

L, C, B, H, W = 4, 256, 8, 32, 32
NH, KD = 8, 64
KH = NH * KD          # 512
HW = H * W            # 1024
P = 128
NC = 8                # cores
TOPK = 4
EPS = 1e-7
BN_EPS = 1e-5
PW = W + 2            # 34
PHW = PW * (H + 2)    # 1156
CHUNKS = [(0, 11), (11, 11), (22, 10)]

f32 = mybir.dt.float32
bf16 = mybir.dt.bfloat16
AX = mybir.AxisListType
OP = mybir.AluOpType
ACTF = mybir.ActivationFunctionType

_compiled = {}


def _build(ncores=NC, layers=L, no_cc=False, dbg=False, dbgl=0):
    nc = bacc.Bacc(None, target_bir_lowering=False, debug=False, num_devices=ncores)

    # ---- DRAM I/O (per-core shapes) ----
    xin = nc.dram_tensor("xin", [C, HW], f32, kind="ExternalInput").ap()
    wkvd = nc.dram_tensor("wkvd", [L, P, 6 * KH], bf16, kind="ExternalInput").ap()
    bkvd = nc.dram_tensor("bkvd", [L, 1, 3 * KH], bf16, kind="ExternalInput").ap()
    w1d = nc.dram_tensor("w1d", [L, 2, P, 36 * P], bf16, kind="ExternalInput").ap()
    w2d = nc.dram_tensor("w2d", [L, 2, P, 18 * P], bf16, kind="ExternalInput").ap()
    cstd = nc.dram_tensor("cstd", [L, P, 8], f32, kind="ExternalInput").ap()
    out = nc.dram_tensor("out", [C, HW], f32, kind="ExternalOutput").ap()
    dbgt = {}
    if dbg:
        for nm, shp in [("d_q", [P, 8 * KH]), ("d_k", [P, 8 * KH]),
                        ("d_v", [P, 8 * KH]), ("d_S", [P, 320]),
                        ("d_attn", [P, 320]), ("d_o", [P, 8 * KH]),
                        ("d_opad0", [P, PHW]), ("d_opad1", [P, PHW]),
                        ("d_y1_0", [P, HW]),
                        ("d_gsum", [P, 4]), ("d_A0", [P, 1]), ("d_B0", [P, 1]),
                        ("d_h1p0", [P, PHW]), ("d_x0", [P, HW])]:
            dbgt[nm] = nc.dram_tensor(nm, shp, f32, kind="ExternalOutput").ap()

    with tile.TileContext(nc) as tc:
        with tc.tile_pool(name="main", bufs=1) as mp, \
             tc.tile_pool(name="prodp", bufs=2) as prodp, \
             tc.tile_pool(name="wp", bufs=1) as wp, \
             tc.tile_pool(name="wkvp", bufs=2) as wkvp, \
             tc.tile_pool(name="cstp", bufs=2) as cstp, \
             tc.tile_pool(name="kqvps", bufs=2, space="PSUM") as kqvps, \
             tc.tile_pool(name="convps", bufs=3, space="PSUM") as convps, \
             tc.tile_pool(name="tpsp", bufs=2, space="PSUM") as tpsp, \
             tc.tile_pool(name="hbp", bufs=1, space="PSUM") as hbp, \
             tc.tile_pool(name="dramp", bufs=2, space="DRAM") as dramp:

            # ---- persistent tiles ----
            x = [mp.tile([P, HW], f32, name=f"x{i}") for i in range(2)]
            xb = [mp.tile([P, HW], bf16, name=f"xb{i}") for i in range(2)]
            qbt = mp.tile([P, 8 * KH], bf16, name="qbt")
            kbt = [mp.tile([P, 8 * KH], bf16, name=f"kbt{i}") for i in range(L)]
            vbt = [mp.tile([P, 8 * KH], bf16, name=f"vbt{i}") for i in range(L)]
            S = mp.tile([P, 64 * 5], f32, name="S")       # [p, t, g] t-major
            attn = mp.tile([P, 64 * 5], f32, name="attn")
            attnb = mp.tile([P, 64 * 5], bf16, name="attnb")
            zs = mp.tile([P, 64], f32, name="zs")
            dmin = mp.tile([P, 64], f32, name="dmin")
            mxp = mp.tile([P, 64], f32, name="mxp")
            pr = mp.tile([P, 8 * KH], bf16, name="pr")    # scores product / wsum tmp
            r1 = mp.tile([P, 2048], bf16, name="r1")
            r2 = mp.tile([P, 1024], bf16, name="r2")
            r3 = mp.tile([P, 512], bf16, name="r3")
            r4 = mp.tile([P, 256], bf16, name="r4")
            r5 = mp.tile([P, 128], bf16, name="r5")
            o = mp.tile([P, 8 * KH], bf16, name="o")
            opad = [mp.tile([P, PHW + 2], bf16, name=f"opad{i}") for i in range(4)]
            y1 = [mp.tile([P, HW], bf16, name=f"y1_{i}") for i in range(2)]
            h1p = [mp.tile([P, PHW + 2], bf16, name=f"h1p{i}") for i in range(2)]
            scr = mp.tile([P, 512], f32, name="scr")      # stats scratch out
            ssum = mp.tile([P, 8], f32, name="ssum")
            ssq = mp.tile([P, 8], f32, name="ssq")
            st = mp.tile([P, 4], f32, name="st")
            gst = mp.tile([P, 32], f32, name="gst")
            gsum = mp.tile([P, 4], f32, name="gsum")
            ones1 = mp.tile([1, P], bf16, name="ones1")
            onesf = mp.tile([1, P], f32, name="onesf")
            ident = mp.tile([P, P], bf16, name="ident")
            # BN coeff scratch
            t1 = [mp.tile([P, 1], f32, name=f"t1_{i}") for i in range(2)]
            Ac = [mp.tile([P, 1], f32, name=f"Ac{i}") for i in range(2)]
            Bc = [mp.tile([P, 1], f32, name=f"Bc{i}") for i in range(2)]
            sq = mp.tile([P, 1], f32, name="sq")
            vart = mp.tile([P, 1], f32, name="vart")
            stdt = mp.tile([P, 1], f32, name="stdt")

            # ---- init ----
            for i in range(2):
                nc.sync.dma_start(x[i][:], xin[i * P:(i + 1) * P, :])
                nc.scalar.copy(xb[i][:], x[i][:])
            for i in range(4):
                nc.vector.memset(opad[i][:], 0)
            for i in range(2):
                nc.vector.memset(h1p[i][:], 0)
            nc.vector.memset(ones1[:], 1.0)
            nc.vector.memset(onesf[:], 1.0)
            nc.vector.memset(S[:], 0)
            nc.vector.memset(attn[:], 0)
            nc.vector.memset(ssum[:], 0)
            nc.vector.memset(ssq[:], 0)
            make_identity(nc, ident[:])

            # views
            pr3 = pr[:].rearrange("p (g i) -> p g i", g=8)     # [p, pb, 512]
            r13 = r1[:].rearrange("p (g i) -> p g i", g=8)
            r23 = r2[:].rearrange("p (g i) -> p g i", g=8)
            r33 = r3[:].rearrange("p (g i) -> p g i", g=8)
            r43 = r4[:].rearrange("p (g i) -> p g i", g=8)
            r53 = r5[:].rearrange("p (g i) -> p g i", g=8)
            St = S[:].rearrange("p (t g) -> p t g", g=64)      # [p, 5, 64]
            at_t = attn[:].rearrange("p (t g) -> p t g", g=64)
            ab_t = attnb[:].rearrange("p (t g) -> p t g", g=64)
            o4 = o[:].rearrange("p (a d h) -> p a d h", a=8, h=8)
            pr4 = pr[:].rearrange("p (a d h) -> p a d h", a=8, h=8)

            hb_ps = hbp.tile([64, 64], f32, name="hb_ps", tag="hb")

            def heartbeat(src_ap):
                # tiny matmul whose rhs depends on fresh DVE output; keeps
                # the PE HAM activity window from going idle.
                lhs = onesf if src_ap.dtype == f32 else ones1
                n = src_ap.shape[-1]
                nc.tensor.matmul(hb_ps[0:64, 0:n], lhs[0:1, 0:64], src_ap,
                                 start=True, stop=True)

            for l in range(layers):
                R = l + 1      # number of real keys
                T = R + 1      # +1 zero key

                # ---- per-layer weight / const loads (one DMA each) ----
                wkvt = wkvp.tile([P, 6 * KH], bf16, name="wkvt", tag="wkv")
                nc.sync.dma_start(wkvt[:], wkvd[l])
                wkv = wkvt[:].rearrange("p (c n) -> p c n", c=2)
                bkv = cstp.tile([1, 3 * KH], bf16, name="bkv", tag="bkv")
                nc.sync.dma_start(bkv[:], bkvd[l])
                cst = cstp.tile([P, 8], f32, name="cst", tag="cst")
                nc.sync.dma_start(cst[:], cstd[l])
                w1s = [wp.tile([P, 36 * P], bf16, name=f"w1s{co}", tag=f"w1s{co}")
                       for co in range(2)]
                for co in range(2):
                    nc.sync.dma_start(w1s[co][:], w1d[l, co])
                w2s = [wp.tile([P, 18 * P], bf16, name=f"w2s{co}", tag=f"w2s{co}")
                       for co in range(2)]
                for co in range(2):
                    nc.sync.dma_start(w2s[co][:], w2d[l, co])

                # ---- K/Q/V 1x1 convs, position-major, d-major channels ----
                for ni, dest in ((0, kbt[l][:]), (1, qbt[:]), (2, vbt[l][:])):
                    for pb in range(8):
                        ps = kqvps.tile([P, KH], f32, name="kqv_ps")
                        nc.tensor.matmul(ps[:], ones1[:],
                                         bkv[:, ni * KH:(ni + 1) * KH],
                                         start=True, stop=False)
                        for ct in range(2):
                            nc.tensor.matmul(
                                ps[:], xb[ct][:, pb * P:(pb + 1) * P],
                                wkv[:, ct, ni * KH:(ni + 1) * KH],
                                start=False, stop=(ct == 1))
                        nc.scalar.copy(dest[:, pb * KH:(pb + 1) * KH], ps[:])

                # ---- scores: S[:, t, :] = sum_d q*k_t  (bf16 tree) ----
                for t in range(R):
                    prt = prodp.tile([P, 8 * KH], bf16, name="prt", tag="prt")
                    prt3 = prt[:].rearrange("p (g i) -> p g i", g=8)
                    nc.vector.tensor_mul(prt[:], qbt[:], kbt[t][:])
                    heartbeat(prt[0:1, 0:64])
                    nc.vector.tensor_tensor(r13, prt3[:, :, 0:256],
                                            prt3[:, :, 256:512], OP.add)
                    nc.vector.tensor_tensor(r23, r13[:, :, 0:128],
                                            r13[:, :, 128:256], OP.add)
                    heartbeat(r2[0:1, 0:64])
                    nc.vector.tensor_tensor(r33, r23[:, :, 0:64],
                                            r23[:, :, 64:128], OP.add)
                    nc.vector.tensor_tensor(r43, r33[:, :, 0:32],
                                            r33[:, :, 32:64], OP.add)
                    nc.vector.tensor_tensor(r53, r43[:, :, 0:16],
                                            r43[:, :, 16:32], OP.add)
                    nc.vector.tensor_tensor(St[:, t, :].rearrange(
                        "p (g i) -> p g i", g=8),
                        r53[:, :, 0:8], r53[:, :, 8:16], OP.add)
                    heartbeat(S[0:1, t * 64:t * 64 + 64])
                nc.vector.memset(St[:, R, :], 0)  # zero-key slot

                # ---- softmax over T slots (scores are small: skip max-sub) ----
                nc.scalar.activation(attn[:, 0:T * 64], S[:, 0:T * 64], ACTF.Exp)
                nc.vector.tensor_reduce(
                    out=zs[:], in_=attn[:, 0:T * 64].rearrange(
                        "p (t g) -> p g t", t=T),
                    axis=AX.X, op=OP.add)
                nc.vector.reciprocal(zs[:], zs[:])
                heartbeat(zs[0:1, 0:64])
                nc.vector.tensor_tensor(
                    at_t[:, 0:T], at_t[:, 0:T],
                    zs[:].unsqueeze(1).broadcast_to([P, T, 64]), OP.mult)

                # ---- sparse top-k (only T=5) ----
                if T > TOPK:
                    first = True
                    for i in range(T):
                        for j in range(i + 1, T):
                            dst = dmin if first else mxp
                            nc.vector.tensor_tensor(
                                dst[:], at_t[:, i], at_t[:, j], OP.max)
                            if not first:
                                nc.vector.tensor_tensor(dmin[:], dmin[:],
                                                        mxp[:], OP.min)
                            first = False
                    heartbeat(dmin[0:1, 0:64])
                    nc.vector.tensor_scalar_add(dmin[:], dmin[:], EPS)
                    nc.vector.tensor_tensor(
                        at_t[:, 0:T], at_t[:, 0:T],
                        dmin[:].unsqueeze(1).broadcast_to([P, T, 64]),
                        OP.subtract)
                    nc.vector.tensor_scalar_max(attn[:, 0:T * 64],
                                                attn[:, 0:T * 64], 0.0)
                    nc.vector.tensor_reduce(
                        out=zs[:], in_=attn[:, 0:T * 64].rearrange(
                            "p (t g) -> p g t", t=T),
                        axis=AX.X, op=OP.add)
                    nc.vector.tensor_scalar_add(zs[:], zs[:], EPS)
                    nc.vector.reciprocal(zs[:], zs[:])
                    heartbeat(zs[0:1, 0:64])
                    nc.vector.tensor_tensor(
                        at_t[:, 0:T], at_t[:, 0:T],
                        zs[:].unsqueeze(1).broadcast_to([P, T, 64]), OP.mult)

                nc.vector.tensor_copy(attnb[:, 0:T * 64], attn[:, 0:T * 64])

                # ---- weighted sum: o = sum_t attn_t * v_t (bf16, 2x APs) ----
                for t in range(R):
                    v4 = vbt[t][:].rearrange("p (a d h) -> p a d h", a=8, h=8)
                    ab4 = ab_t[:, t].rearrange("p (a h) -> p a h", a=8) \
                        .unsqueeze(2).broadcast_to([P, 8, KD, 8])
                    if t == 0:
                        nc.vector.tensor_tensor(o4, v4, ab4, OP.mult)
                    else:
                        nc.vector.tensor_tensor(pr4, v4, ab4, OP.mult)
                        heartbeat(pr[0:1, 0:64])
                        nc.vector.tensor_add(o[:], o[:], pr[:])
                    heartbeat(o[0:1, 0:64])

                # ---- transpose o -> opad (channel-major, padded) ----
                for pb in range(8):
                    for ci in range(4):
                        tps = tpsp.tile([P, P], bf16, name="tps", tag="tps")
                        nc.tensor.transpose(
                            tps[:], o[:, pb * KH + ci * P:pb * KH + ci * P + P],
                            ident[:])
                        opv = opad[ci][:, 0:PHW].rearrange("c (i j) -> c i j",
                                                           j=PW)
                        nc.scalar.copy(
                            opv[:, 1 + 4 * pb:5 + 4 * pb, 1:W + 1],
                            tps[:].rearrange("c (r w) -> c r w", w=W))

                # ---- conv3x3 #1 (bf16) + per-co BN stats/AllGather/h1 ----
                # co=0's AllGather is issued at conv1 midpoint and hides
                # under co=1's matmuls; co=1's is partially hidden by the
                # ci=0 half of conv2 below.
                w1v = [w1s[co][:].rearrange("p (t c j) -> p t c j", t=9, c=4)
                       for co in range(2)]
                NTOT = float(ncores * HW)
                for co in range(2):
                    for ck, (i0, nr) in enumerate(CHUNKS):
                        ps = convps.tile([P, 512], f32, name="c1ps", tag="cps")
                        nw = PW * nr
                        for tap in range(9):
                            ty, tx = tap // 3, tap % 3
                            base = PW * (i0 + ty) + tx
                            for ci in range(4):
                                nc.tensor.matmul(
                                    ps[:, 0:nw], w1v[co][:, tap, ci],
                                    opad[ci][:, base:base + nw],
                                    start=(tap == 0 and ci == 0),
                                    stop=(tap == 8 and ci == 3))
                        ysl = y1[co][:, W * i0:W * (i0 + nr)]
                        nc.scalar.copy(
                            ysl.rearrange("c (i j) -> c i j", j=W),
                            ps[:, 0:nw].rearrange("c (i j) -> c i j",
                                                  j=PW)[:, :, 0:W])
                        # BN stats for this chunk (hidden under conv matmuls)
                        nc.vector.scalar_tensor_tensor(
                            out=scr[:, 0:W * nr], in0=ysl, scalar=1.0,
                            in1=ysl, op0=OP.mult, op1=OP.mult,
                            accum_out=ssq[:, 4 * co + ck:4 * co + ck + 1])
                        nc.vector.tensor_reduce(
                            out=ssum[:, 4 * co + ck:4 * co + ck + 1],
                            in_=ysl, axis=AX.X, op=OP.add)
                    # per-co stats total + AllGather
                    nc.vector.tensor_reduce(
                        out=st[:, 2 * co:2 * co + 1],
                        in_=ssum[:, 4 * co:4 * co + 4], axis=AX.X, op=OP.add)
                    nc.vector.tensor_reduce(
                        out=st[:, 2 * co + 1:2 * co + 2],
                        in_=ssq[:, 4 * co:4 * co + 4], axis=AX.X, op=OP.add)
                    if no_cc:
                        nc.vector.tensor_scalar_mul(
                            gsum[:, 2 * co:2 * co + 2],
                            st[:, 2 * co:2 * co + 2], float(ncores))
                    else:
                        cci = dramp.tile([1, 256], f32, name="cci",
                                         tag=f"cci{co}")
                        cco = dramp.tile([ncores, 256], f32, name="cco",
                                         tag=f"cco{co}", addr_space="Shared")
                        nc.sync.dma_start(
                            cci[0].rearrange("(p j) -> p j", j=2),
                            st[:, 2 * co:2 * co + 2])
                        nc.gpsimd.collective_compute(
                            "AllGather", OP.bypass,
                            replica_groups=[list(range(ncores))],
                            ins=[cci.opt()], outs=[cco.opt()])
                        nc.sync.dma_start(
                            gst[:, co * 16:co * 16 + 2 * ncores].rearrange(
                                "p (j s) -> p j s", s=ncores),
                            cco[:].rearrange("s (p j) -> p j s", j=2))
                        nc.vector.tensor_reduce(
                            out=gsum[:, 2 * co:2 * co + 2],
                            in_=gst[:, co * 16:co * 16 + 2 * ncores].rearrange(
                                "p (j s) -> p j s", s=ncores),
                            axis=AX.X, op=OP.add)
                        heartbeat(gst[0:1, co * 16:co * 16 + 16])
                    # BN coefficients: A = g/sqrt(var+eps), B = b - mean*A
                    nc.vector.tensor_scalar_mul(t1[co][:],
                                                gsum[:, 2 * co:2 * co + 1],
                                                1.0 / NTOT)
                    nc.vector.tensor_scalar_mul(vart[:],
                                                gsum[:, 2 * co + 1:2 * co + 2],
                                                1.0 / NTOT)
                    nc.vector.tensor_mul(sq[:], t1[co][:], t1[co][:])
                    nc.vector.tensor_sub(vart[:], vart[:], sq[:])
                    nc.vector.tensor_scalar_add(vart[:], vart[:], BN_EPS)
                    nc.scalar.activation(stdt[:], vart[:], ACTF.Sqrt)
                    nc.vector.reciprocal(stdt[:], stdt[:])
                    nc.vector.tensor_mul(Ac[co][:], cst[:, co:co + 1], stdt[:])
                    nc.vector.tensor_mul(sq[:], t1[co][:], Ac[co][:])
                    nc.vector.tensor_sub(Bc[co][:], cst[:, 2 + co:3 + co],
                                         sq[:])
                    heartbeat(Bc[co][0:1, 0:1])
                    # h1 = relu(A*y1 + B) into padded conv2 input (bf16)
                    h1v = h1p[co][:, 0:PHW].rearrange("c (i j) -> c i j", j=PW)
                    nc.scalar.activation(
                        h1v[:, 1:H + 1, 1:W + 1],
                        y1[co][:].rearrange("c (i j) -> c i j", j=W),
                        ACTF.Relu, bias=Bc[co][:], scale=Ac[co][:])

                # ---- conv3x3 #2 (bf16) + residual x += gamma*h2 ----
                # co_out=0 accumulates its ci=0 taps first: those depend only
                # on h1p[0] (BN co=0) and stream while AllGather co=1 is in
                # flight; the ci=1 taps close the PSUM groups after BN co=1.
                w2v = [w2s[co][:].rearrange("p (t c j) -> p t c j", t=9, c=2)
                       for co in range(2)]
                for co in range(2):
                    nc.scalar.add(x[co][:], x[co][:], cst[:, 4 + co:5 + co])

                def conv2_chunk_half(ps, co, i0, nr, ci, start):
                    nw = PW * nr
                    for tap in range(9):
                        ty, tx = tap // 3, tap % 3
                        base = PW * (i0 + ty) + tx
                        nc.tensor.matmul(
                            ps[:, 0:nw], w2v[co][:, tap, ci],
                            h1p[ci][:, base:base + nw],
                            start=(start and tap == 0),
                            stop=(not start and tap == 8))

                def conv2_residual(ps, co, i0, nr):
                    nw = PW * nr
                    xslice = x[co][:, W * i0:W * (i0 + nr)]
                    nc.vector.scalar_tensor_tensor(
                        out=xslice.rearrange("c (i j) -> c i j", j=W),
                        in0=ps[:, 0:nw].rearrange("c (i j) -> c i j",
                                                  j=PW)[:, :, 0:W],
                        scalar=cst[:, 6:7],
                        in1=xslice.rearrange("c (i j) -> c i j", j=W),
                        op0=OP.mult, op1=OP.add)

                cps0 = []
                for (i0, nr) in CHUNKS:
                    ps = convps.tile([P, 512], f32, name="c2ps", tag="cps")
                    conv2_chunk_half(ps, 0, i0, nr, 0, True)
                    cps0.append(ps)
                for ck, (i0, nr) in enumerate(CHUNKS):
                    conv2_chunk_half(cps0[ck], 0, i0, nr, 1, False)
                    conv2_residual(cps0[ck], 0, i0, nr)
                for (i0, nr) in CHUNKS:
                    ps = convps.tile([P, 512], f32, name="c2ps", tag="cps")
                    conv2_chunk_half(ps, 1, i0, nr, 0, True)
                    conv2_chunk_half(ps, 1, i0, nr, 1, False)
                    conv2_residual(ps, 1, i0, nr)
                for co in range(2):
                    if l < layers - 1:
                        nc.scalar.copy(xb[co][:], x[co][:])
                    else:
                        nc.sync.dma_start(out[co * P:(co + 1) * P, :], x[co][:])
                if dbg and l == dbgl:
                    def dump(dst, srct, n):
                        for c0 in range(0, n, 2048):
                            cw = min(2048, n - c0)
                            fc = prodp.tile([P, 2048], f32, name="fcvt",
                                            tag="prt")
                            nc.vector.tensor_copy(fc[:, 0:cw],
                                                  srct[:, c0:c0 + cw])
                            nc.sync.dma_start(dst[:, c0:c0 + cw], fc[:, 0:cw])
                    for nm, srct in [("d_q", qbt), ("d_k", kbt[l]),
                                     ("d_v", vbt[l]), ("d_o", o)]:
                        dump(dbgt[nm], srct[:], 8 * KH)
                    nc.sync.dma_start(dbgt["d_S"], S[:])
                    nc.sync.dma_start(dbgt["d_attn"], attn[:])
                    for ci in range(2):
                        dump(dbgt[f"d_opad{ci}"], opad[ci][:], PHW)
                    dump(dbgt["d_y1_0"], y1[0][:], HW)
                    nc.sync.dma_start(dbgt["d_gsum"], gsum[:])
                    nc.sync.dma_start(dbgt["d_A0"], Ac[0][:])
                    nc.sync.dma_start(dbgt["d_B0"], Bc[0][:])
                    dump(dbgt["d_h1p0"], h1p[0][:], PHW)
                    nc.sync.dma_start(dbgt["d_x0"], x[0][:])

    nc.compile()
    return nc


def _host_prep(inputs):
    bf = ml_dtypes.bfloat16
    kw, kb, qw, qb = inputs["kw"], inputs["kb"], inputs["qw"], inputs["qb"]
    vw, vb = inputs["vw"], inputs["vb"]
    ow1, ow2 = inputs["ow1"], inputs["ow2"]
    gammas, ob2 = inputs["gammas"], inputs["ob2"]

    # d-major channel permutation: new col dh -> old col h*64+d
    dh = np.arange(KH)
    perm = (dh % NH) * KD + dh // NH

    def packw(wm):  # [L, KH, C] -> [L, 2, 128, KH] with d-major cols
        return wm.transpose(0, 2, 1)[:, :, perm].reshape(L, 2, P, KH)

    d = {}
    wkv = np.concatenate([packw(kw), packw(qw / 8.0), packw(vw)], axis=3)
    # kernel-side tile is [P, (chunk, col)] -> reorder [L, 2, P, 1536] to
    # [L, P, 2, 1536] before flattening
    d["wkvd"] = np.ascontiguousarray(
        wkv.transpose(0, 2, 1, 3).reshape(L, P, 6 * KH)).astype(bf)
    bkv = np.concatenate([kb[:, perm], (qb / 8.0)[:, perm], vb[:, perm]],
                         axis=1).reshape(L, 1, 3 * KH)
    d["bkvd"] = np.ascontiguousarray(bkv).astype(bf)

    # conv1 weights: [L, co, p(cin in transposed-o order), tap, ci, jo]
    # transposed-o partition p of chtile ci holds original v-channel
    # vh = (p%8)*64 + ci*16 + p//8
    ow1r = ow1.reshape(L, 2, P, KH, 3, 3)  # [l, co, jo, vh, ty, tx]
    w1 = np.empty((L, 2, P, 9, 4, P), np.float32)
    j = np.arange(P)
    for ci in range(4):
        vh = (j % 8) * 64 + ci * 16 + j // 8
        sub = ow1r[:, :, :, vh, :, :]          # [l, co, jo, p, ty, tx]
        w1[:, :, :, :, ci, :] = sub.transpose(0, 1, 3, 4, 5, 2).reshape(
            L, 2, P, 9, P)
    d["w1d"] = np.ascontiguousarray(w1.reshape(L, 2, P, 36 * P)).astype(bf)

    # conv2 weights: [L, co, p(cin), tap, ci, jo]
    a2 = ow2.reshape(L, 2, P, 2, P, 3, 3)      # [l, co, jo, ci, p, ty, tx]
    w2 = a2.transpose(0, 1, 4, 5, 6, 3, 2).reshape(L, 2, P, 9, 2, P)
    d["w2d"] = np.ascontiguousarray(w2.reshape(L, 2, P, 18 * P)).astype(bf)

    # per-layer consts: [bng0, bng1, bnb0, bnb1, gob0, gob1, gam, 0]
    cst = np.zeros((L, P, 8), np.float32)
    bn_g, bn_b = inputs["bn_g"], inputs["bn_b"]
    gob2 = gammas[:, None] * ob2
    for co in range(2):
        cst[:, :, co] = bn_g[:, co * P:(co + 1) * P]
        cst[:, :, 2 + co] = bn_b[:, co * P:(co + 1) * P]
        cst[:, :, 4 + co] = gob2[:, co * P:(co + 1) * P]
    cst[:, :, 6] = gammas[:, None]
    d["cstd"] = np.ascontiguousarray(cst)
    return d


def _in_maps(inputs):
    shared = _host_prep(inputs)
    x = np.ascontiguousarray(inputs["x"].reshape(B, C, HW)).astype(np.float32)
    in_maps = []
    for c in range(NC):
        m = dict(shared)
        m["xin"] = x[c]
        in_maps.append(m)
    return in_maps


def kernel(**inputs):
    if "nc" not in _compiled:
        _compiled["nc"] = _build()
    nc = _compiled["nc"]
    in_maps = _in_maps(inputs)
    res = bass_utils.run_bass_kernel_spmd(nc, in_maps, core_ids=list(range(NC)))
    outs = np.stack([res.results[c]["out"] for c in range(NC)])
    return outs.reshape(B, C, H, W).astype(np.float32)


# revision 10
# speedup vs baseline: 2.5073x; 1.0350x over previous
"""AttentiveDensenet Trainium2 Bass kernel (v2).

Data-parallel over batch B=8 across 8 NeuronCores (1 image per core).

Key design points (v2, driven by the v1 HW trace):
  - Conv weights are host-packed into one contiguous DRAM block per
    (layer, conv, co-half) and fetched with a single large DMA well ahead
    of use, so conv matmuls stream back-to-back (v1 issued 36 small weight
    DMAs per chunk and the PE starved, degrading each MM to isolated+cold
    timing).
  - Channel order for q/k/v is d-major (col = d*8 + head) so the score
    d-reduction is a contiguous-halving tree of bf16 tensor_tensor adds
    (2x DVE mode) instead of a 1x tensor_reduce, and the attn-weighted
    v-sum multiplies with a step-1-innermost broadcast AP (2x) instead of
    a step-0 broadcast (1x).
  - o is accumulated in bf16 and transposed to channel-major with 32
    PE-transposes straight into the padded conv input (v1 bounced o
    through DRAM + xbar-transpose, ~25-45us/layer of serial DMA).
  - BN stats (sum, sum-sq) are computed per conv1 chunk (hidden under
    conv1's matmuls) via scalar_tensor_tensor accum_out; only the 2KB
    AllGather + coefficient math + h1 remain exposed.
  - Tiny "heartbeat" matmuls are threaded through the attention/BN
    phases (each depending on a fresh DVE result) so the PE's HAM clock
    gate never sees a >3.4us idle window and matmuls stay at 2.4 GHz.
  - conv chunks are (11, 11, 10) rows so every matmul has N>=340 and
    LDWEIGHTS (~107ns) hides under the matmul (~150ns); v1's (15,15,2)
    left a 68-wide chunk that ran LDWEIGHTS-bound.
"""
import numpy as np
import ml_dtypes

import concourse.bacc as bacc
import concourse.mybir as mybir
import concourse.tile as tile
from concourse import bass_utils
from concourse.masks import make_identity

L, C, B, H, W = 4, 256, 8, 32, 32
NH, KD = 8, 64
KH = NH * KD          # 512
HW = H * W            # 1024
P = 128
NC = 8                # cores
TOPK = 4
EPS = 1e-7
BN_EPS = 1e-5
PW = W + 2            # 34
PHW = PW * (H + 2)    # 1156
CHUNKS = [(0, 11), (11, 11), (22, 10)]

f32 = mybir.dt.float32
bf16 = mybir.dt.bfloat16
AX = mybir.AxisListType
OP = mybir.AluOpType
ACTF = mybir.ActivationFunctionType

_compiled = {}


def _build(ncores=NC, layers=L, no_cc=False, dbg=False, dbgl=0):
    nc = bacc.Bacc(None, target_bir_lowering=False, debug=False, num_devices=ncores)

    # ---- DRAM I/O (per-core shapes) ----
    xin = nc.dram_tensor("xin", [C, HW], f32, kind="ExternalInput").ap()
    wkvd = nc.dram_tensor("wkvd", [L, P, 6 * KH], bf16, kind="ExternalInput").ap()
    bkvd = nc.dram_tensor("bkvd", [L, 1, 3 * KH], bf16, kind="ExternalInput").ap()
    w1d = nc.dram_tensor("w1d", [L, 2, P, 36 * P], bf16, kind="ExternalInput").ap()
    w2d = nc.dram_tensor("w2d", [L, 2, P, 18 * P], bf16, kind="ExternalInput").ap()
    cstd = nc.dram_tensor("cstd", [L, P, 8], f32, kind="ExternalInput").ap()
    out = nc.dram_tensor("out", [C, HW], f32, kind="ExternalOutput").ap()
    dbgt = {}
    if dbg:
        for nm, shp in [("d_q", [P, 8 * KH]), ("d_k", [P, 8 * KH]),
                        ("d_v", [P, 8 * KH]), ("d_S", [P, 320]),
                        ("d_attn", [P, 320]), ("d_o", [P, 8 * KH]),
                        ("d_opad0", [P, PHW]), ("d_opad1", [P, PHW]),
                        ("d_y1_0", [P, HW]),
                        ("d_gsum", [P, 4]), ("d_A0", [P, 1]), ("d_B0", [P, 1]),
                        ("d_h1p0", [P, PHW]), ("d_x0", [P, HW])]:
            dbgt[nm] = nc.dram_tensor(nm, shp, f32, kind="ExternalOutput").ap()

    with tile.TileContext(nc) as tc:
        with tc.tile_pool(name="main", bufs=1) as mp, \
             tc.tile_pool(name="prodp", bufs=2) as prodp, \
             tc.tile_pool(name="wp", bufs=1) as wp, \
             tc.tile_pool(name="wkvp", bufs=2) as wkvp, \
             tc.tile_pool(name="cstp", bufs=2) as cstp, \
             tc.tile_pool(name="kqvps", bufs=2, space="PSUM") as kqvps, \
             tc.tile_pool(name="convps", bufs=3, space="PSUM") as convps, \
             tc.tile_pool(name="tpsp", bufs=2, space="PSUM") as tpsp, \
             tc.tile_pool(name="hbp", bufs=1, space="PSUM") as hbp, \
             tc.tile_pool(name="dramp", bufs=2, space="DRAM") as dramp:

            # ---- persistent tiles ----
            x = [mp.tile([P, HW], f32, name=f"x{i}") for i in range(2)]
            xb = [mp.tile([P, HW], bf16, name=f"xb{i}") for i in range(2)]
            qbt = mp.tile([P, 8 * KH], bf16, name="qbt")
            kbt = [mp.tile([P, 8 * KH], bf16, name=f"kbt{i}") for i in range(L)]
            vbt = [mp.tile([P, 8 * KH], bf16, name=f"vbt{i}") for i in range(L)]
            S = mp.tile([P, 64 * 5], f32, name="S")       # [p, t, g] t-major
            attn = mp.tile([P, 64 * 5], f32, name="attn")
            attnb = mp.tile([P, 64 * 5], bf16, name="attnb")
            zs = mp.tile([P, 64], f32, name="zs")
            dmin = mp.tile([P, 64], f32, name="dmin")
            mxp = mp.tile([P, 64], f32, name="mxp")
            pr = mp.tile([P, 8 * KH], bf16, name="pr")    # scores product / wsum tmp
            r1 = mp.tile([P, 2048], bf16, name="r1")
            r2 = mp.tile([P, 1024], bf16, name="r2")
            r3 = mp.tile([P, 512], bf16, name="r3")
            r4 = mp.tile([P, 256], bf16, name="r4")
            r5 = mp.tile([P, 128], bf16, name="r5")
            o = mp.tile([P, 8 * KH], bf16, name="o")
            opad = [mp.tile([P, PHW + 2], bf16, name=f"opad{i}") for i in range(4)]
            y1 = [mp.tile([P, HW], bf16, name=f"y1_{i}") for i in range(2)]
            h1p = [mp.tile([P, PHW + 2], bf16, name=f"h1p{i}") for i in range(2)]
            scr = mp.tile([P, 512], f32, name="scr")      # stats scratch out
            ssum = mp.tile([P, 8], f32, name="ssum")
            ssq = mp.tile([P, 8], f32, name="ssq")
            st = mp.tile([P, 4], f32, name="st")
            stT = mp.tile([2, P], f32, name="stT")
            ccs = mp.tile([NC, 256], f32, name="ccs")
            gsum = mp.tile([P, 4], f32, name="gsum")
            ones1 = mp.tile([1, P], bf16, name="ones1")
            onesf = mp.tile([1, P], f32, name="onesf")
            ident = mp.tile([P, P], bf16, name="ident")
            identF = mp.tile([P, P], f32, name="identF")
            # BN coeff scratch
            t1 = [mp.tile([P, 1], f32, name=f"t1_{i}") for i in range(2)]
            Ac = [mp.tile([P, 1], f32, name=f"Ac{i}") for i in range(2)]
            Bc = [mp.tile([P, 1], f32, name=f"Bc{i}") for i in range(2)]
            sq = mp.tile([P, 1], f32, name="sq")
            vart = mp.tile([P, 1], f32, name="vart")
            stdt = mp.tile([P, 1], f32, name="stdt")

            # ---- init ----
            for i in range(2):
                nc.sync.dma_start(x[i][:], xin[i * P:(i + 1) * P, :])
                nc.scalar.copy(xb[i][:], x[i][:])
            for i in range(4):
                nc.vector.memset(opad[i][:], 0)
            for i in range(2):
                nc.vector.memset(h1p[i][:], 0)
            nc.vector.memset(ones1[:], 1.0)
            nc.vector.memset(onesf[:], 1.0)
            nc.vector.memset(S[:], 0)
            nc.vector.memset(attn[:], 0)
            nc.vector.memset(ssum[:], 0)
            nc.vector.memset(ssq[:], 0)
            make_identity(nc, ident[:])
            make_identity(nc, identF[:])

            # views
            pr3 = pr[:].rearrange("p (g i) -> p g i", g=8)     # [p, pb, 512]
            r13 = r1[:].rearrange("p (g i) -> p g i", g=8)
            r23 = r2[:].rearrange("p (g i) -> p g i", g=8)
            r33 = r3[:].rearrange("p (g i) -> p g i", g=8)
            r43 = r4[:].rearrange("p (g i) -> p g i", g=8)
            r53 = r5[:].rearrange("p (g i) -> p g i", g=8)
            St = S[:].rearrange("p (t g) -> p t g", g=64)      # [p, 5, 64]
            at_t = attn[:].rearrange("p (t g) -> p t g", g=64)
            ab_t = attnb[:].rearrange("p (t g) -> p t g", g=64)
            o4 = o[:].rearrange("p (a d h) -> p a d h", a=8, h=8)
            pr4 = pr[:].rearrange("p (a d h) -> p a d h", a=8, h=8)

            hb_ps = hbp.tile([64, 64], f32, name="hb_ps", tag="hb")

            def heartbeat(src_ap):
                # tiny matmul whose rhs depends on fresh DVE output; keeps
                # the PE HAM activity window from going idle.
                lhs = onesf if src_ap.dtype == f32 else ones1
                n = src_ap.shape[-1]
                nc.tensor.matmul(hb_ps[0:64, 0:n], lhs[0:1, 0:64], src_ap,
                                 start=True, stop=True)

            for l in range(layers):
                R = l + 1      # number of real keys
                T = R + 1      # +1 zero key

                # ---- per-layer weight / const loads (one DMA each) ----
                wkvt = wkvp.tile([P, 6 * KH], bf16, name="wkvt", tag="wkv")
                nc.sync.dma_start(wkvt[:], wkvd[l])
                wkv = wkvt[:].rearrange("p (c n) -> p c n", c=2)
                bkv = cstp.tile([1, 3 * KH], bf16, name="bkv", tag="bkv")
                nc.sync.dma_start(bkv[:], bkvd[l])
                cst = cstp.tile([P, 8], f32, name="cst", tag="cst")
                nc.sync.dma_start(cst[:], cstd[l])
                w1s = [wp.tile([P, 36 * P], bf16, name=f"w1s{co}", tag=f"w1s{co}")
                       for co in range(2)]
                for co in range(2):
                    nc.sync.dma_start(w1s[co][:], w1d[l, co])
                w2s = [wp.tile([P, 18 * P], bf16, name=f"w2s{co}", tag=f"w2s{co}")
                       for co in range(2)]
                for co in range(2):
                    nc.sync.dma_start(w2s[co][:], w2d[l, co])

                # ---- K/Q/V 1x1 convs, position-major, d-major channels ----
                for ni, dest in ((0, kbt[l][:]), (1, qbt[:]), (2, vbt[l][:])):
                    for pb in range(8):
                        ps = kqvps.tile([P, KH], f32, name="kqv_ps")
                        nc.tensor.matmul(ps[:], ones1[:],
                                         bkv[:, ni * KH:(ni + 1) * KH],
                                         start=True, stop=False)
                        for ct in range(2):
                            nc.tensor.matmul(
                                ps[:], xb[ct][:, pb * P:(pb + 1) * P],
                                wkv[:, ct, ni * KH:(ni + 1) * KH],
                                start=False, stop=(ct == 1))
                        nc.scalar.copy(dest[:, pb * KH:(pb + 1) * KH], ps[:])

                # ---- scores: S[:, t, :] = sum_d q*k_t  (bf16 tree) ----
                for t in range(R):
                    prt = prodp.tile([P, 8 * KH], bf16, name="prt", tag="prt")
                    prt3 = prt[:].rearrange("p (g i) -> p g i", g=8)
                    nc.vector.tensor_mul(prt[:], qbt[:], kbt[t][:])
                    heartbeat(prt[0:1, 0:64])
                    nc.vector.tensor_tensor(r13, prt3[:, :, 0:256],
                                            prt3[:, :, 256:512], OP.add)
                    nc.vector.tensor_tensor(r23, r13[:, :, 0:128],
                                            r13[:, :, 128:256], OP.add)
                    heartbeat(r2[0:1, 0:64])
                    nc.vector.tensor_tensor(r33, r23[:, :, 0:64],
                                            r23[:, :, 64:128], OP.add)
                    nc.vector.tensor_tensor(r43, r33[:, :, 0:32],
                                            r33[:, :, 32:64], OP.add)
                    nc.vector.tensor_tensor(r53, r43[:, :, 0:16],
                                            r43[:, :, 16:32], OP.add)
                    nc.vector.tensor_tensor(St[:, t, :].rearrange(
                        "p (g i) -> p g i", g=8),
                        r53[:, :, 0:8], r53[:, :, 8:16], OP.add)
                    heartbeat(S[0:1, t * 64:t * 64 + 64])
                nc.vector.memset(St[:, R, :], 0)  # zero-key slot

                # ---- softmax over T slots (scores are small: skip max-sub) ----
                nc.scalar.activation(attn[:, 0:T * 64], S[:, 0:T * 64], ACTF.Exp)
                nc.vector.tensor_reduce(
                    out=zs[:], in_=attn[:, 0:T * 64].rearrange(
                        "p (t g) -> p g t", t=T),
                    axis=AX.X, op=OP.add)
                nc.vector.reciprocal(zs[:], zs[:])
                heartbeat(zs[0:1, 0:64])
                nc.vector.tensor_tensor(
                    at_t[:, 0:T], at_t[:, 0:T],
                    zs[:].unsqueeze(1).broadcast_to([P, T, 64]), OP.mult)

                # ---- sparse top-k (only T=5) ----
                if T > TOPK:
                    first = True
                    for i in range(T):
                        for j in range(i + 1, T):
                            dst = dmin if first else mxp
                            nc.vector.tensor_tensor(
                                dst[:], at_t[:, i], at_t[:, j], OP.max)
                            if not first:
                                nc.vector.tensor_tensor(dmin[:], dmin[:],
                                                        mxp[:], OP.min)
                            first = False
                    heartbeat(dmin[0:1, 0:64])
                    nc.vector.tensor_scalar_add(dmin[:], dmin[:], EPS)
                    nc.vector.tensor_tensor(
                        at_t[:, 0:T], at_t[:, 0:T],
                        dmin[:].unsqueeze(1).broadcast_to([P, T, 64]),
                        OP.subtract)
                    nc.vector.tensor_scalar_max(attn[:, 0:T * 64],
                                                attn[:, 0:T * 64], 0.0)
                    nc.vector.tensor_reduce(
                        out=zs[:], in_=attn[:, 0:T * 64].rearrange(
                            "p (t g) -> p g t", t=T),
                        axis=AX.X, op=OP.add)
                    nc.vector.tensor_scalar_add(zs[:], zs[:], EPS)
                    nc.vector.reciprocal(zs[:], zs[:])
                    heartbeat(zs[0:1, 0:64])
                    nc.vector.tensor_tensor(
                        at_t[:, 0:T], at_t[:, 0:T],
                        zs[:].unsqueeze(1).broadcast_to([P, T, 64]), OP.mult)

                nc.vector.tensor_copy(attnb[:, 0:T * 64], attn[:, 0:T * 64])

                # ---- weighted sum: o = sum_t attn_t * v_t (bf16, 2x APs) ----
                for t in range(R):
                    v4 = vbt[t][:].rearrange("p (a d h) -> p a d h", a=8, h=8)
                    ab4 = ab_t[:, t].rearrange("p (a h) -> p a h", a=8) \
                        .unsqueeze(2).broadcast_to([P, 8, KD, 8])
                    if t == 0:
                        nc.vector.tensor_tensor(o4, v4, ab4, OP.mult)
                    else:
                        nc.vector.tensor_tensor(pr4, v4, ab4, OP.mult)
                        heartbeat(pr[0:1, 0:64])
                        nc.vector.tensor_add(o[:], o[:], pr[:])
                    heartbeat(o[0:1, 0:64])

                # ---- transpose o -> opad (channel-major, padded) ----
                for pb in range(8):
                    for ci in range(4):
                        tps = tpsp.tile([P, P], bf16, name="tps", tag="tps")
                        nc.tensor.transpose(
                            tps[:], o[:, pb * KH + ci * P:pb * KH + ci * P + P],
                            ident[:])
                        opv = opad[ci][:, 0:PHW].rearrange("c (i j) -> c i j",
                                                           j=PW)
                        nc.scalar.copy(
                            opv[:, 1 + 4 * pb:5 + 4 * pb, 1:W + 1],
                            tps[:].rearrange("c (r w) -> c r w", w=W))

                # ---- conv3x3 #1 (bf16) + per-co BN stats/AllGather/h1 ----
                # co=0's AllGather is issued at conv1 midpoint and hides
                # under co=1's matmuls; co=1's is partially hidden by the
                # ci=0 half of conv2 below.
                w1v = [w1s[co][:].rearrange("p (t c j) -> p t c j", t=9, c=4)
                       for co in range(2)]
                NTOT = float(ncores * HW)
                for co in range(2):
                    for ck, (i0, nr) in enumerate(CHUNKS):
                        ps = convps.tile([P, 512], f32, name="c1ps", tag="cps")
                        nw = PW * nr
                        for tap in range(9):
                            ty, tx = tap // 3, tap % 3
                            base = PW * (i0 + ty) + tx
                            for ci in range(4):
                                nc.tensor.matmul(
                                    ps[:, 0:nw], w1v[co][:, tap, ci],
                                    opad[ci][:, base:base + nw],
                                    start=(tap == 0 and ci == 0),
                                    stop=(tap == 8 and ci == 3))
                        ysl = y1[co][:, W * i0:W * (i0 + nr)]
                        nc.scalar.copy(
                            ysl.rearrange("c (i j) -> c i j", j=W),
                            ps[:, 0:nw].rearrange("c (i j) -> c i j",
                                                  j=PW)[:, :, 0:W])
                        # BN stats for this chunk (hidden under conv matmuls)
                        nc.vector.scalar_tensor_tensor(
                            out=scr[:, 0:W * nr], in0=ysl, scalar=1.0,
                            in1=ysl, op0=OP.mult, op1=OP.mult,
                            accum_out=ssq[:, 4 * co + ck:4 * co + ck + 1])
                        nc.vector.tensor_reduce(
                            out=ssum[:, 4 * co + ck:4 * co + ck + 1],
                            in_=ysl, axis=AX.X, op=OP.add)
                    # per-co stats total + AllGather
                    nc.vector.tensor_reduce(
                        out=st[:, 2 * co:2 * co + 1],
                        in_=ssum[:, 4 * co:4 * co + 4], axis=AX.X, op=OP.add)
                    nc.vector.tensor_reduce(
                        out=st[:, 2 * co + 1:2 * co + 2],
                        in_=ssq[:, 4 * co:4 * co + 4], axis=AX.X, op=OP.add)
                    if no_cc:
                        nc.vector.tensor_scalar_mul(
                            gsum[:, 2 * co:2 * co + 2],
                            st[:, 2 * co:2 * co + 2], float(ncores))
                    else:
                        # pack [128,2] stats onto 2 partitions so the DRAM
                        # DMAs use 2 big descriptors instead of 128 tiny ones
                        stp = tpsp.tile([2, P], f32, name="stp", tag="tps")
                        nc.tensor.transpose(stp[:], st[:, 2 * co:2 * co + 2],
                                            identF[:])
                        nc.scalar.copy(stT[:], stp[:])
                        cci = dramp.tile([1, 256], f32, name="cci",
                                         tag=f"cci{co}")
                        cco = dramp.tile([ncores, 256], f32, name="cco",
                                         tag=f"cco{co}", addr_space="Shared")
                        nc.sync.dma_start(
                            cci[0].rearrange("(p j) -> p j", p=2), stT[:])
                        nc.gpsimd.collective_compute(
                            "AllGather", OP.bypass,
                            replica_groups=[list(range(ncores))],
                            ins=[cci.opt()], outs=[cco.opt()])
                        nc.sync.dma_start(ccs[:], cco[:])
                        heartbeat(ccs[0:1, 0:64])
                        # unpack: transpose [8,128] core-major slabs back to
                        # per-partition and reduce over cores
                        for j in range(2):
                            tpa = tpsp.tile([P, NC], f32, name="tpa",
                                            tag="tps")
                            nc.tensor.transpose(
                                tpa[:], ccs[:, j * P:(j + 1) * P],
                                identF[0:NC, 0:NC])
                            nc.vector.tensor_reduce(
                                out=gsum[:, 2 * co + j:2 * co + j + 1],
                                in_=tpa[:], axis=AX.X, op=OP.add)
                    # BN coefficients: A = g/sqrt(var+eps), B = b - mean*A
                    nc.vector.tensor_scalar_mul(t1[co][:],
                                                gsum[:, 2 * co:2 * co + 1],
                                                1.0 / NTOT)
                    nc.vector.tensor_scalar_mul(vart[:],
                                                gsum[:, 2 * co + 1:2 * co + 2],
                                                1.0 / NTOT)
                    nc.vector.tensor_mul(sq[:], t1[co][:], t1[co][:])
                    nc.vector.tensor_sub(vart[:], vart[:], sq[:])
                    nc.vector.tensor_scalar_add(vart[:], vart[:], BN_EPS)
                    nc.scalar.activation(stdt[:], vart[:], ACTF.Sqrt)
                    nc.vector.reciprocal(stdt[:], stdt[:])
                    nc.vector.tensor_mul(Ac[co][:], cst[:, co:co + 1], stdt[:])
                    nc.vector.tensor_mul(sq[:], t1[co][:], Ac[co][:])
                    nc.vector.tensor_sub(Bc[co][:], cst[:, 2 + co:3 + co],
                                         sq[:])
                    heartbeat(Bc[co][0:1, 0:1])
                    # h1 = relu(A*y1 + B) into padded conv2 input (bf16)
                    h1v = h1p[co][:, 0:PHW].rearrange("c (i j) -> c i j", j=PW)
                    nc.scalar.activation(
                        h1v[:, 1:H + 1, 1:W + 1],
                        y1[co][:].rearrange("c (i j) -> c i j", j=W),
                        ACTF.Relu, bias=Bc[co][:], scale=Ac[co][:])

                # ---- conv3x3 #2 (bf16) + residual x += gamma*h2 ----
                # co_out=0 accumulates its ci=0 taps first: those depend only
                # on h1p[0] (BN co=0) and stream while AllGather co=1 is in
                # flight; the ci=1 taps close the PSUM groups after BN co=1.
                w2v = [w2s[co][:].rearrange("p (t c j) -> p t c j", t=9, c=2)
                       for co in range(2)]
                for co in range(2):
                    nc.scalar.add(x[co][:], x[co][:], cst[:, 4 + co:5 + co])

                def conv2_chunk_half(ps, co, i0, nr, ci, start):
                    nw = PW * nr
                    for tap in range(9):
                        ty, tx = tap // 3, tap % 3
                        base = PW * (i0 + ty) + tx
                        nc.tensor.matmul(
                            ps[:, 0:nw], w2v[co][:, tap, ci],
                            h1p[ci][:, base:base + nw],
                            start=(start and tap == 0),
                            stop=(not start and tap == 8))

                def conv2_residual(ps, co, i0, nr):
                    nw = PW * nr
                    xslice = x[co][:, W * i0:W * (i0 + nr)]
                    nc.vector.scalar_tensor_tensor(
                        out=xslice.rearrange("c (i j) -> c i j", j=W),
                        in0=ps[:, 0:nw].rearrange("c (i j) -> c i j",
                                                  j=PW)[:, :, 0:W],
                        scalar=cst[:, 6:7],
                        in1=xslice.rearrange("c (i j) -> c i j", j=W),
                        op0=OP.mult, op1=OP.add)

                cps0 = []
                for (i0, nr) in CHUNKS:
                    ps = convps.tile([P, 512], f32, name="c2ps", tag="cps")
                    conv2_chunk_half(ps, 0, i0, nr, 0, True)
                    cps0.append(ps)
                for ck, (i0, nr) in enumerate(CHUNKS):
                    conv2_chunk_half(cps0[ck], 0, i0, nr, 1, False)
                    conv2_residual(cps0[ck], 0, i0, nr)
                for (i0, nr) in CHUNKS:
                    ps = convps.tile([P, 512], f32, name="c2ps", tag="cps")
                    conv2_chunk_half(ps, 1, i0, nr, 0, True)
                    conv2_chunk_half(ps, 1, i0, nr, 1, False)
                    conv2_residual(ps, 1, i0, nr)
                for co in range(2):
                    if l < layers - 1:
                        nc.scalar.copy(xb[co][:], x[co][:])
                    else:
                        nc.sync.dma_start(out[co * P:(co + 1) * P, :], x[co][:])
                if dbg and l == dbgl:
                    def dump(dst, srct, n):
                        for c0 in range(0, n, 2048):
                            cw = min(2048, n - c0)
                            fc = prodp.tile([P, 2048], f32, name="fcvt",
                                            tag="prt")
                            nc.vector.tensor_copy(fc[:, 0:cw],
                                                  srct[:, c0:c0 + cw])
                            nc.sync.dma_start(dst[:, c0:c0 + cw], fc[:, 0:cw])
                    for nm, srct in [("d_q", qbt), ("d_k", kbt[l]),
                                     ("d_v", vbt[l]), ("d_o", o)]:
                        dump(dbgt[nm], srct[:], 8 * KH)
                    nc.sync.dma_start(dbgt["d_S"], S[:])
                    nc.sync.dma_start(dbgt["d_attn"], attn[:])
                    for ci in range(2):
                        dump(dbgt[f"d_opad{ci}"], opad[ci][:], PHW)
                    dump(dbgt["d_y1_0"], y1[0][:], HW)
                    nc.sync.dma_start(dbgt["d_gsum"], gsum[:])
                    nc.sync.dma_start(dbgt["d_A0"], Ac[0][:])
                    nc.sync.dma_start(dbgt["d_B0"], Bc[0][:])
                    dump(dbgt["d_h1p0"], h1p[0][:], PHW)
                    nc.sync.dma_start(dbgt["d_x0"], x[0][:])

    nc.compile()
    return nc


def _host_prep(inputs):
    bf = ml_dtypes.bfloat16
    kw, kb, qw, qb = inputs["kw"], inputs["kb"], inputs["qw"], inputs["qb"]
    vw, vb = inputs["vw"], inputs["vb"]
    ow1, ow2 = inputs["ow1"], inputs["ow2"]
    gammas, ob2 = inputs["gammas"], inputs["ob2"]

    # d-major channel permutation: new col dh -> old col h*64+d
    dh = np.arange(KH)
    perm = (dh % NH) * KD + dh // NH

    def packw(wm):  # [L, KH, C] -> [L, 2, 128, KH] with d-major cols
        return wm.transpose(0, 2, 1)[:, :, perm].reshape(L, 2, P, KH)

    d = {}
    wkv = np.concatenate([packw(kw), packw(qw / 8.0), packw(vw)], axis=3)
    # kernel-side tile is [P, (chunk, col)] -> reorder [L, 2, P, 1536] to
    # [L, P, 2, 1536] before flattening
    d["wkvd"] = np.ascontiguousarray(
        wkv.transpose(0, 2, 1, 3).reshape(L, P, 6 * KH)).astype(bf)
    bkv = np.concatenate([kb[:, perm], (qb / 8.0)[:, perm], vb[:, perm]],
                         axis=1).reshape(L, 1, 3 * KH)
    d["bkvd"] = np.ascontiguousarray(bkv).astype(bf)

    # conv1 weights: [L, co, p(cin in transposed-o order), tap, ci, jo]
    # transposed-o partition p of chtile ci holds original v-channel
    # vh = (p%8)*64 + ci*16 + p//8
    ow1r = ow1.reshape(L, 2, P, KH, 3, 3)  # [l, co, jo, vh, ty, tx]
    w1 = np.empty((L, 2, P, 9, 4, P), np.float32)
    j = np.arange(P)
    for ci in range(4):
        vh = (j % 8) * 64 + ci * 16 + j // 8
        sub = ow1r[:, :, :, vh, :, :]          # [l, co, jo, p, ty, tx]
        w1[:, :, :, :, ci, :] = sub.transpose(0, 1, 3, 4, 5, 2).reshape(
            L, 2, P, 9, P)
    d["w1d"] = np.ascontiguousarray(w1.reshape(L, 2, P, 36 * P)).astype(bf)

    # conv2 weights: [L, co, p(cin), tap, ci, jo]
    a2 = ow2.reshape(L, 2, P, 2, P, 3, 3)      # [l, co, jo, ci, p, ty, tx]
    w2 = a2.transpose(0, 1, 4, 5, 6, 3, 2).reshape(L, 2, P, 9, 2, P)
    d["w2d"] = np.ascontiguousarray(w2.reshape(L, 2, P, 18 * P)).astype(bf)

    # per-layer consts: [bng0, bng1, bnb0, bnb1, gob0, gob1, gam, 0]
    cst = np.zeros((L, P, 8), np.float32)
    bn_g, bn_b = inputs["bn_g"], inputs["bn_b"]
    gob2 = gammas[:, None] * ob2
    for co in range(2):
        cst[:, :, co] = bn_g[:, co * P:(co + 1) * P]
        cst[:, :, 2 + co] = bn_b[:, co * P:(co + 1) * P]
        cst[:, :, 4 + co] = gob2[:, co * P:(co + 1) * P]
    cst[:, :, 6] = gammas[:, None]
    d["cstd"] = np.ascontiguousarray(cst)
    return d


def _in_maps(inputs):
    shared = _host_prep(inputs)
    x = np.ascontiguousarray(inputs["x"].reshape(B, C, HW)).astype(np.float32)
    in_maps = []
    for c in range(NC):
        m = dict(shared)
        m["xin"] = x[c]
        in_maps.append(m)
    return in_maps


def kernel(**inputs):
    if "nc" not in _compiled:
        _compiled["nc"] = _build()
    nc = _compiled["nc"]
    in_maps = _in_maps(inputs)
    res = bass_utils.run_bass_kernel_spmd(nc, in_maps, core_ids=list(range(NC)))
    outs = np.stack([res.results[c]["out"] for c in range(NC)])
    return outs.reshape(B, C, H, W).astype(np.float32)


# revision 12
# speedup vs baseline: 2.5622x; 1.0219x over previous
"""AttentiveDensenet Trainium2 Bass kernel (v2).

Data-parallel over batch B=8 across 8 NeuronCores (1 image per core).

Key design points (v2, driven by the v1 HW trace):
  - Conv weights are host-packed into one contiguous DRAM block per
    (layer, conv, co-half) and fetched with a single large DMA well ahead
    of use, so conv matmuls stream back-to-back (v1 issued 36 small weight
    DMAs per chunk and the PE starved, degrading each MM to isolated+cold
    timing).
  - Channel order for q/k/v is d-major (col = d*8 + head) so the score
    d-reduction is a contiguous-halving tree of bf16 tensor_tensor adds
    (2x DVE mode) instead of a 1x tensor_reduce, and the attn-weighted
    v-sum multiplies with a step-1-innermost broadcast AP (2x) instead of
    a step-0 broadcast (1x).
  - o is accumulated in bf16 and transposed to channel-major with 32
    PE-transposes straight into the padded conv input (v1 bounced o
    through DRAM + xbar-transpose, ~25-45us/layer of serial DMA).
  - BN stats (sum, sum-sq) are computed per conv1 chunk (hidden under
    conv1's matmuls) via scalar_tensor_tensor accum_out; only the 2KB
    AllGather + coefficient math + h1 remain exposed.
  - Tiny "heartbeat" matmuls are threaded through the attention/BN
    phases (each depending on a fresh DVE result) so the PE's HAM clock
    gate never sees a >3.4us idle window and matmuls stay at 2.4 GHz.
  - conv chunks are (11, 11, 10) rows so every matmul has N>=340 and
    LDWEIGHTS (~107ns) hides under the matmul (~150ns); v1's (15,15,2)
    left a 68-wide chunk that ran LDWEIGHTS-bound.
"""
import numpy as np
import ml_dtypes

import concourse.bacc as bacc
import concourse.mybir as mybir
import concourse.tile as tile
from concourse import bass_utils
from concourse.masks import make_identity

L, C, B, H, W = 4, 256, 8, 32, 32
NH, KD = 8, 64
KH = NH * KD          # 512
HW = H * W            # 1024
P = 128
NC = 8                # cores
TOPK = 4
EPS = 1e-7
BN_EPS = 1e-5
PW = W + 2            # 34
PHW = PW * (H + 2)    # 1156
CHUNKS = [(0, 11), (11, 11), (22, 10)]

f32 = mybir.dt.float32
bf16 = mybir.dt.bfloat16
AX = mybir.AxisListType
OP = mybir.AluOpType
ACTF = mybir.ActivationFunctionType

_compiled = {}


def _build(ncores=NC, layers=L, no_cc=False, dbg=False, dbgl=0):
    nc = bacc.Bacc(None, target_bir_lowering=False, debug=False, num_devices=ncores)

    # ---- DRAM I/O (per-core shapes) ----
    xin = nc.dram_tensor("xin", [C, HW], f32, kind="ExternalInput").ap()
    wkvd = nc.dram_tensor("wkvd", [L, P, 6 * KH], bf16, kind="ExternalInput").ap()
    bkvd = nc.dram_tensor("bkvd", [L, 1, 3 * KH], bf16, kind="ExternalInput").ap()
    w1d = nc.dram_tensor("w1d", [L, 2, P, 36 * P], bf16, kind="ExternalInput").ap()
    w2d = nc.dram_tensor("w2d", [L, 2, P, 18 * P], bf16, kind="ExternalInput").ap()
    cstd = nc.dram_tensor("cstd", [L, P, 8], f32, kind="ExternalInput").ap()
    out = nc.dram_tensor("out", [C, HW], f32, kind="ExternalOutput").ap()
    dbgt = {}
    if dbg:
        for nm, shp in [("d_q", [P, 8 * KH]), ("d_k", [P, 8 * KH]),
                        ("d_v", [P, 8 * KH]), ("d_S", [P, 320]),
                        ("d_attn", [P, 320]), ("d_o", [P, 8 * KH]),
                        ("d_opad0", [P, PHW]), ("d_opad1", [P, PHW]),
                        ("d_y1_0", [P, HW]),
                        ("d_gsum", [P, 4]), ("d_A0", [P, 1]), ("d_B0", [P, 1]),
                        ("d_h1p0", [P, PHW]), ("d_x0", [P, HW])]:
            dbgt[nm] = nc.dram_tensor(nm, shp, f32, kind="ExternalOutput").ap()

    with tile.TileContext(nc) as tc:
        with tc.tile_pool(name="main", bufs=1) as mp, \
             tc.tile_pool(name="prodp", bufs=2) as prodp, \
             tc.tile_pool(name="wp", bufs=1) as wp, \
             tc.tile_pool(name="wkvp", bufs=2) as wkvp, \
             tc.tile_pool(name="cstp", bufs=2) as cstp, \
             tc.tile_pool(name="kqvps", bufs=2, space="PSUM") as kqvps, \
             tc.tile_pool(name="convps", bufs=3, space="PSUM") as convps, \
             tc.tile_pool(name="tpsp", bufs=2, space="PSUM") as tpsp, \
             tc.tile_pool(name="hbp", bufs=1, space="PSUM") as hbp, \
             tc.tile_pool(name="dramp", bufs=2, space="DRAM") as dramp:

            # ---- persistent tiles ----
            x = [mp.tile([P, HW], f32, name=f"x{i}") for i in range(2)]
            xb = [mp.tile([P, HW], bf16, name=f"xb{i}") for i in range(2)]
            qbt = mp.tile([P, 8 * KH], bf16, name="qbt")
            kbt = [mp.tile([P, 8 * KH], bf16, name=f"kbt{i}") for i in range(L)]
            vbt = [mp.tile([P, 8 * KH], bf16, name=f"vbt{i}") for i in range(L)]
            S = mp.tile([P, 64 * 5], f32, name="S")       # [p, t, g] t-major
            attn = mp.tile([P, 64 * 5], f32, name="attn")
            attnb = mp.tile([P, 64 * 5], bf16, name="attnb")
            zs = mp.tile([P, 64], f32, name="zs")
            dmin = mp.tile([P, 64], f32, name="dmin")
            mxp = mp.tile([P, 64], f32, name="mxp")
            pr = mp.tile([P, 8 * KH], bf16, name="pr")    # scores product / wsum tmp
            r1 = mp.tile([P, 2048], bf16, name="r1")
            r2 = mp.tile([P, 1024], bf16, name="r2")
            r3 = mp.tile([P, 512], bf16, name="r3")
            r4 = mp.tile([P, 256], bf16, name="r4")
            r5 = mp.tile([P, 128], bf16, name="r5")
            o = mp.tile([P, 8 * KH], bf16, name="o")
            opad = [mp.tile([P, PHW + 2], bf16, name=f"opad{i}") for i in range(4)]
            y1 = [mp.tile([P, HW], bf16, name=f"y1_{i}") for i in range(2)]
            h1p = [mp.tile([P, PHW + 2], bf16, name=f"h1p{i}") for i in range(2)]
            scr = mp.tile([P, 512], f32, name="scr")      # stats scratch out
            ssum = mp.tile([P, 8], f32, name="ssum")
            ssq = mp.tile([P, 8], f32, name="ssq")
            st = mp.tile([P, 4], f32, name="st")
            stT = mp.tile([2, P], f32, name="stT")
            ccs = mp.tile([NC, 256], f32, name="ccs")
            gsum = mp.tile([P, 4], f32, name="gsum")
            ones1 = mp.tile([1, P], bf16, name="ones1")
            onesf = mp.tile([1, P], f32, name="onesf")
            ident = mp.tile([P, P], bf16, name="ident")
            identF = mp.tile([P, P], f32, name="identF")
            # BN coeff scratch
            t1 = [mp.tile([P, 1], f32, name=f"t1_{i}") for i in range(2)]
            Ac = [mp.tile([P, 1], f32, name=f"Ac{i}") for i in range(2)]
            Bc = [mp.tile([P, 1], f32, name=f"Bc{i}") for i in range(2)]
            sq = mp.tile([P, 1], f32, name="sq")
            vart = mp.tile([P, 1], f32, name="vart")
            stdt = mp.tile([P, 1], f32, name="stdt")

            # ---- init ----
            for i in range(2):
                nc.sync.dma_start(x[i][:], xin[i * P:(i + 1) * P, :])
                nc.scalar.copy(xb[i][:], x[i][:])
            for i in range(4):
                nc.vector.memset(opad[i][:], 0)
            for i in range(2):
                nc.vector.memset(h1p[i][:], 0)
            nc.vector.memset(ones1[:], 1.0)
            nc.vector.memset(onesf[:], 1.0)
            nc.vector.memset(S[:], 0)
            nc.vector.memset(attn[:], 0)
            nc.vector.memset(ssum[:], 0)
            nc.vector.memset(ssq[:], 0)
            make_identity(nc, ident[:])
            make_identity(nc, identF[:])
            if not no_cc:
                # startup sync: absorb the inter-core launch stagger and
                # warm the CC mesh while layer-0 kqv/attention runs, so the
                # first real AllGather doesn't eat ~17us of entry skew.
                cciw = dramp.tile([1, 8], f32, name="cciw")
                ccow = dramp.tile([ncores, 8], f32, name="ccow",
                                  addr_space="Shared")
                nc.sync.dma_start(cciw[:], onesf[0:1, 0:8])
                nc.gpsimd.collective_compute(
                    "AllGather", OP.bypass,
                    replica_groups=[list(range(ncores))],
                    ins=[cciw.opt()], outs=[ccow.opt()])

            # views
            pr3 = pr[:].rearrange("p (g i) -> p g i", g=8)     # [p, pb, 512]
            r13 = r1[:].rearrange("p (g i) -> p g i", g=8)
            r23 = r2[:].rearrange("p (g i) -> p g i", g=8)
            r33 = r3[:].rearrange("p (g i) -> p g i", g=8)
            r43 = r4[:].rearrange("p (g i) -> p g i", g=8)
            r53 = r5[:].rearrange("p (g i) -> p g i", g=8)
            St = S[:].rearrange("p (t g) -> p t g", g=64)      # [p, 5, 64]
            at_t = attn[:].rearrange("p (t g) -> p t g", g=64)
            ab_t = attnb[:].rearrange("p (t g) -> p t g", g=64)
            o4 = o[:].rearrange("p (a d h) -> p a d h", a=8, h=8)
            pr4 = pr[:].rearrange("p (a d h) -> p a d h", a=8, h=8)

            hb_ps = hbp.tile([64, 64], f32, name="hb_ps", tag="hb")

            def heartbeat(src_ap):
                # tiny matmul whose rhs depends on fresh DVE output; keeps
                # the PE HAM activity window from going idle.
                lhs = onesf if src_ap.dtype == f32 else ones1
                n = src_ap.shape[-1]
                nc.tensor.matmul(hb_ps[0:64, 0:n], lhs[0:1, 0:64], src_ap,
                                 start=True, stop=True)

            for l in range(layers):
                R = l + 1      # number of real keys
                T = R + 1      # +1 zero key

                # ---- per-layer weight / const loads (one DMA each) ----
                wkvt = wkvp.tile([P, 6 * KH], bf16, name="wkvt", tag="wkv")
                nc.sync.dma_start(wkvt[:], wkvd[l])
                wkv = wkvt[:].rearrange("p (c n) -> p c n", c=2)
                bkv = cstp.tile([1, 3 * KH], bf16, name="bkv", tag="bkv")
                nc.sync.dma_start(bkv[:], bkvd[l])
                cst = cstp.tile([P, 8], f32, name="cst", tag="cst")
                nc.sync.dma_start(cst[:], cstd[l])
                w1s = [wp.tile([P, 36 * P], bf16, name=f"w1s{co}", tag=f"w1s{co}")
                       for co in range(2)]
                for co in range(2):
                    nc.sync.dma_start(w1s[co][:], w1d[l, co])
                w2s = [wp.tile([P, 18 * P], bf16, name=f"w2s{co}", tag=f"w2s{co}")
                       for co in range(2)]
                for co in range(2):
                    nc.sync.dma_start(w2s[co][:], w2d[l, co])

                # ---- K/Q/V 1x1 convs, position-major, d-major channels ----
                for ni, dest in ((0, kbt[l][:]), (1, qbt[:]), (2, vbt[l][:])):
                    for pb in range(8):
                        ps = kqvps.tile([P, KH], f32, name="kqv_ps")
                        nc.tensor.matmul(ps[:], ones1[:],
                                         bkv[:, ni * KH:(ni + 1) * KH],
                                         start=True, stop=False)
                        for ct in range(2):
                            nc.tensor.matmul(
                                ps[:], xb[ct][:, pb * P:(pb + 1) * P],
                                wkv[:, ct, ni * KH:(ni + 1) * KH],
                                start=False, stop=(ct == 1))
                        nc.scalar.copy(dest[:, pb * KH:(pb + 1) * KH], ps[:])

                # ---- scores: S[:, t, :] = sum_d q*k_t  (bf16 tree) ----
                for t in range(R):
                    prt = prodp.tile([P, 8 * KH], bf16, name="prt", tag="prt")
                    prt3 = prt[:].rearrange("p (g i) -> p g i", g=8)
                    nc.vector.tensor_mul(prt[:], qbt[:], kbt[t][:])
                    heartbeat(prt[0:1, 0:64])
                    nc.vector.tensor_tensor(r13, prt3[:, :, 0:256],
                                            prt3[:, :, 256:512], OP.add)
                    nc.vector.tensor_tensor(r23, r13[:, :, 0:128],
                                            r13[:, :, 128:256], OP.add)
                    heartbeat(r2[0:1, 0:64])
                    nc.vector.tensor_tensor(r33, r23[:, :, 0:64],
                                            r23[:, :, 64:128], OP.add)
                    nc.vector.tensor_tensor(r43, r33[:, :, 0:32],
                                            r33[:, :, 32:64], OP.add)
                    nc.vector.tensor_tensor(r53, r43[:, :, 0:16],
                                            r43[:, :, 16:32], OP.add)
                    nc.vector.tensor_tensor(St[:, t, :].rearrange(
                        "p (g i) -> p g i", g=8),
                        r53[:, :, 0:8], r53[:, :, 8:16], OP.add)
                    heartbeat(S[0:1, t * 64:t * 64 + 64])
                nc.vector.memset(St[:, R, :], 0)  # zero-key slot

                # ---- softmax over T slots (scores are small: skip max-sub) ----
                nc.scalar.activation(attn[:, 0:T * 64], S[:, 0:T * 64], ACTF.Exp)
                nc.vector.tensor_reduce(
                    out=zs[:], in_=attn[:, 0:T * 64].rearrange(
                        "p (t g) -> p g t", t=T),
                    axis=AX.X, op=OP.add)
                nc.vector.reciprocal(zs[:], zs[:])
                heartbeat(zs[0:1, 0:64])
                nc.vector.tensor_tensor(
                    at_t[:, 0:T], at_t[:, 0:T],
                    zs[:].unsqueeze(1).broadcast_to([P, T, 64]), OP.mult)

                # ---- sparse top-k (only T=5) ----
                if T > TOPK:
                    first = True
                    for i in range(T):
                        for j in range(i + 1, T):
                            dst = dmin if first else mxp
                            nc.vector.tensor_tensor(
                                dst[:], at_t[:, i], at_t[:, j], OP.max)
                            if not first:
                                nc.vector.tensor_tensor(dmin[:], dmin[:],
                                                        mxp[:], OP.min)
                            first = False
                    heartbeat(dmin[0:1, 0:64])
                    nc.vector.tensor_scalar_add(dmin[:], dmin[:], EPS)
                    nc.vector.tensor_tensor(
                        at_t[:, 0:T], at_t[:, 0:T],
                        dmin[:].unsqueeze(1).broadcast_to([P, T, 64]),
                        OP.subtract)
                    nc.vector.tensor_scalar_max(attn[:, 0:T * 64],
                                                attn[:, 0:T * 64], 0.0)
                    nc.vector.tensor_reduce(
                        out=zs[:], in_=attn[:, 0:T * 64].rearrange(
                            "p (t g) -> p g t", t=T),
                        axis=AX.X, op=OP.add)
                    nc.vector.tensor_scalar_add(zs[:], zs[:], EPS)
                    nc.vector.reciprocal(zs[:], zs[:])
                    heartbeat(zs[0:1, 0:64])
                    nc.vector.tensor_tensor(
                        at_t[:, 0:T], at_t[:, 0:T],
                        zs[:].unsqueeze(1).broadcast_to([P, T, 64]), OP.mult)

                nc.vector.tensor_copy(attnb[:, 0:T * 64], attn[:, 0:T * 64])

                # ---- weighted sum + transpose, split in position halves so
                # half-0's PE transposes (and conv1 chunk 0, via subtile deps)
                # overlap half-1's DVE work ----
                for a0, a1 in ((0, 4), (4, 8)):
                    na = a1 - a0
                    osl = o[:, a0 * KH:a1 * KH]
                    psl = pr[:, a0 * KH:a1 * KH]
                    o4h = osl.rearrange("p (a d h) -> p a d h", a=na, h=8)
                    pr4h = psl.rearrange("p (a d h) -> p a d h", a=na, h=8)
                    for t in range(R):
                        v4 = vbt[t][:, a0 * KH:a1 * KH].rearrange(
                            "p (a d h) -> p a d h", a=na, h=8)
                        ab4 = ab_t[:, t].rearrange(
                            "p (a h) -> p a h", a=8)[:, a0:a1] \
                            .unsqueeze(2).broadcast_to([P, na, KD, 8])
                        if t == 0:
                            nc.vector.tensor_tensor(o4h, v4, ab4, OP.mult)
                        else:
                            nc.vector.tensor_tensor(pr4h, v4, ab4, OP.mult)
                            heartbeat(psl[0:1, 0:64])
                            nc.vector.tensor_add(osl, osl, psl)
                        heartbeat(osl[0:1, 0:64])
                    for pb in range(a0, a1):
                        for ci in range(4):
                            tps = tpsp.tile([P, P], bf16, name="tps", tag="tps")
                            nc.tensor.transpose(
                                tps[:],
                                o[:, pb * KH + ci * P:pb * KH + ci * P + P],
                                ident[:])
                            opv = opad[ci][:, 0:PHW].rearrange(
                                "c (i j) -> c i j", j=PW)
                            nc.scalar.copy(
                                opv[:, 1 + 4 * pb:5 + 4 * pb, 1:W + 1],
                                tps[:].rearrange("c (r w) -> c r w", w=W))

                # ---- conv3x3 #1 (bf16) + per-co BN stats/AllGather/h1 ----
                # co=0's AllGather is issued at conv1 midpoint and hides
                # under co=1's matmuls; co=1's is partially hidden by the
                # ci=0 half of conv2 below.
                w1v = [w1s[co][:].rearrange("p (t c j) -> p t c j", t=9, c=4)
                       for co in range(2)]
                NTOT = float(ncores * HW)
                for co in range(2):
                    for ck, (i0, nr) in enumerate(CHUNKS):
                        ps = convps.tile([P, 512], f32, name="c1ps", tag="cps")
                        nw = PW * nr
                        for tap in range(9):
                            ty, tx = tap // 3, tap % 3
                            base = PW * (i0 + ty) + tx
                            for ci in range(4):
                                nc.tensor.matmul(
                                    ps[:, 0:nw], w1v[co][:, tap, ci],
                                    opad[ci][:, base:base + nw],
                                    start=(tap == 0 and ci == 0),
                                    stop=(tap == 8 and ci == 3))
                        ysl = y1[co][:, W * i0:W * (i0 + nr)]
                        nc.scalar.copy(
                            ysl.rearrange("c (i j) -> c i j", j=W),
                            ps[:, 0:nw].rearrange("c (i j) -> c i j",
                                                  j=PW)[:, :, 0:W])
                        # BN stats for this chunk (hidden under conv matmuls)
                        nc.vector.scalar_tensor_tensor(
                            out=scr[:, 0:W * nr], in0=ysl, scalar=1.0,
                            in1=ysl, op0=OP.mult, op1=OP.mult,
                            accum_out=ssq[:, 4 * co + ck:4 * co + ck + 1])
                        nc.vector.tensor_reduce(
                            out=ssum[:, 4 * co + ck:4 * co + ck + 1],
                            in_=ysl, axis=AX.X, op=OP.add)
                    # per-co stats total + AllGather
                    nc.vector.tensor_reduce(
                        out=st[:, 2 * co:2 * co + 1],
                        in_=ssum[:, 4 * co:4 * co + 4], axis=AX.X, op=OP.add)
                    nc.vector.tensor_reduce(
                        out=st[:, 2 * co + 1:2 * co + 2],
                        in_=ssq[:, 4 * co:4 * co + 4], axis=AX.X, op=OP.add)
                    if no_cc:
                        nc.vector.tensor_scalar_mul(
                            gsum[:, 2 * co:2 * co + 2],
                            st[:, 2 * co:2 * co + 2], float(ncores))
                    else:
                        # pack [128,2] stats onto 2 partitions so the DRAM
                        # DMAs use 2 big descriptors instead of 128 tiny ones
                        stp = tpsp.tile([2, P], f32, name="stp", tag="tps")
                        nc.tensor.transpose(stp[:], st[:, 2 * co:2 * co + 2],
                                            identF[:])
                        nc.scalar.copy(stT[:], stp[:])
                        cci = dramp.tile([1, 256], f32, name="cci",
                                         tag=f"cci{co}")
                        cco = dramp.tile([ncores, 256], f32, name="cco",
                                         tag=f"cco{co}", addr_space="Shared")
                        nc.sync.dma_start(
                            cci[0].rearrange("(p j) -> p j", p=2), stT[:])
                        nc.gpsimd.collective_compute(
                            "AllGather", OP.bypass,
                            replica_groups=[list(range(ncores))],
                            ins=[cci.opt()], outs=[cco.opt()])
                        nc.sync.dma_start(ccs[:], cco[:])
                        heartbeat(ccs[0:1, 0:64])
                        # unpack: transpose [8,128] core-major slabs back to
                        # per-partition and reduce over cores
                        for j in range(2):
                            tpa = tpsp.tile([P, NC], f32, name="tpa",
                                            tag="tps")
                            nc.tensor.transpose(
                                tpa[:], ccs[:, j * P:(j + 1) * P],
                                identF[0:NC, 0:NC])
                            nc.vector.tensor_reduce(
                                out=gsum[:, 2 * co + j:2 * co + j + 1],
                                in_=tpa[:], axis=AX.X, op=OP.add)
                    # BN coefficients: A = g/sqrt(var+eps), B = b - mean*A
                    nc.vector.tensor_scalar_mul(t1[co][:],
                                                gsum[:, 2 * co:2 * co + 1],
                                                1.0 / NTOT)
                    nc.vector.tensor_scalar_mul(vart[:],
                                                gsum[:, 2 * co + 1:2 * co + 2],
                                                1.0 / NTOT)
                    nc.vector.tensor_mul(sq[:], t1[co][:], t1[co][:])
                    nc.vector.tensor_sub(vart[:], vart[:], sq[:])
                    nc.vector.tensor_scalar_add(vart[:], vart[:], BN_EPS)
                    nc.scalar.activation(stdt[:], vart[:], ACTF.Sqrt)
                    nc.vector.reciprocal(stdt[:], stdt[:])
                    nc.vector.tensor_mul(Ac[co][:], cst[:, co:co + 1], stdt[:])
                    nc.vector.tensor_mul(sq[:], t1[co][:], Ac[co][:])
                    nc.vector.tensor_sub(Bc[co][:], cst[:, 2 + co:3 + co],
                                         sq[:])
                    heartbeat(Bc[co][0:1, 0:1])
                    # h1 = relu(A*y1 + B) into padded conv2 input (bf16)
                    h1v = h1p[co][:, 0:PHW].rearrange("c (i j) -> c i j", j=PW)
                    nc.scalar.activation(
                        h1v[:, 1:H + 1, 1:W + 1],
                        y1[co][:].rearrange("c (i j) -> c i j", j=W),
                        ACTF.Relu, bias=Bc[co][:], scale=Ac[co][:])

                # ---- conv3x3 #2 (bf16) + residual x += gamma*h2 ----
                # co_out=0 accumulates its ci=0 taps first: those depend only
                # on h1p[0] (BN co=0) and stream while AllGather co=1 is in
                # flight; the ci=1 taps close the PSUM groups after BN co=1.
                w2v = [w2s[co][:].rearrange("p (t c j) -> p t c j", t=9, c=2)
                       for co in range(2)]
                for co in range(2):
                    nc.scalar.add(x[co][:], x[co][:], cst[:, 4 + co:5 + co])

                def conv2_chunk_half(ps, co, i0, nr, ci, start):
                    nw = PW * nr
                    for tap in range(9):
                        ty, tx = tap // 3, tap % 3
                        base = PW * (i0 + ty) + tx
                        nc.tensor.matmul(
                            ps[:, 0:nw], w2v[co][:, tap, ci],
                            h1p[ci][:, base:base + nw],
                            start=(start and tap == 0),
                            stop=(not start and tap == 8))

                def conv2_residual(ps, co, i0, nr):
                    nw = PW * nr
                    xslice = x[co][:, W * i0:W * (i0 + nr)]
                    nc.vector.scalar_tensor_tensor(
                        out=xslice.rearrange("c (i j) -> c i j", j=W),
                        in0=ps[:, 0:nw].rearrange("c (i j) -> c i j",
                                                  j=PW)[:, :, 0:W],
                        scalar=cst[:, 6:7],
                        in1=xslice.rearrange("c (i j) -> c i j", j=W),
                        op0=OP.mult, op1=OP.add)

                cps0 = []
                for (i0, nr) in CHUNKS:
                    ps = convps.tile([P, 512], f32, name="c2ps", tag="cps")
                    conv2_chunk_half(ps, 0, i0, nr, 0, True)
                    cps0.append(ps)
                for ck, (i0, nr) in enumerate(CHUNKS):
                    conv2_chunk_half(cps0[ck], 0, i0, nr, 1, False)
                    conv2_residual(cps0[ck], 0, i0, nr)
                for (i0, nr) in CHUNKS:
                    ps = convps.tile([P, 512], f32, name="c2ps", tag="cps")
                    conv2_chunk_half(ps, 1, i0, nr, 0, True)
                    conv2_chunk_half(ps, 1, i0, nr, 1, False)
                    conv2_residual(ps, 1, i0, nr)
                for co in range(2):
                    if l < layers - 1:
                        nc.scalar.copy(xb[co][:], x[co][:])
                    else:
                        nc.sync.dma_start(out[co * P:(co + 1) * P, :], x[co][:])
                if dbg and l == dbgl:
                    def dump(dst, srct, n):
                        for c0 in range(0, n, 2048):
                            cw = min(2048, n - c0)
                            fc = prodp.tile([P, 2048], f32, name="fcvt",
                                            tag="prt")
                            nc.vector.tensor_copy(fc[:, 0:cw],
                                                  srct[:, c0:c0 + cw])
                            nc.sync.dma_start(dst[:, c0:c0 + cw], fc[:, 0:cw])
                    for nm, srct in [("d_q", qbt), ("d_k", kbt[l]),
                                     ("d_v", vbt[l]), ("d_o", o)]:
                        dump(dbgt[nm], srct[:], 8 * KH)
                    nc.sync.dma_start(dbgt["d_S"], S[:])
                    nc.sync.dma_start(dbgt["d_attn"], attn[:])
                    for ci in range(2):
                        dump(dbgt[f"d_opad{ci}"], opad[ci][:], PHW)
                    dump(dbgt["d_y1_0"], y1[0][:], HW)
                    nc.sync.dma_start(dbgt["d_gsum"], gsum[:])
                    nc.sync.dma_start(dbgt["d_A0"], Ac[0][:])
                    nc.sync.dma_start(dbgt["d_B0"], Bc[0][:])
                    dump(dbgt["d_h1p0"], h1p[0][:], PHW)
                    nc.sync.dma_start(dbgt["d_x0"], x[0][:])

    nc.compile()
    return nc


def _host_prep(inputs):
    bf = ml_dtypes.bfloat16
    kw, kb, qw, qb = inputs["kw"], inputs["kb"], inputs["qw"], inputs["qb"]
    vw, vb = inputs["vw"], inputs["vb"]
    ow1, ow2 = inputs["ow1"], inputs["ow2"]
    gammas, ob2 = inputs["gammas"], inputs["ob2"]

    # d-major channel permutation: new col dh -> old col h*64+d
    dh = np.arange(KH)
    perm = (dh % NH) * KD + dh // NH

    def packw(wm):  # [L, KH, C] -> [L, 2, 128, KH] with d-major cols
        return wm.transpose(0, 2, 1)[:, :, perm].reshape(L, 2, P, KH)

    d = {}
    wkv = np.concatenate([packw(kw), packw(qw / 8.0), packw(vw)], axis=3)
    # kernel-side tile is [P, (chunk, col)] -> reorder [L, 2, P, 1536] to
    # [L, P, 2, 1536] before flattening
    d["wkvd"] = np.ascontiguousarray(
        wkv.transpose(0, 2, 1, 3).reshape(L, P, 6 * KH)).astype(bf)
    bkv = np.concatenate([kb[:, perm], (qb / 8.0)[:, perm], vb[:, perm]],
                         axis=1).reshape(L, 1, 3 * KH)
    d["bkvd"] = np.ascontiguousarray(bkv).astype(bf)

    # conv1 weights: [L, co, p(cin in transposed-o order), tap, ci, jo]
    # transposed-o partition p of chtile ci holds original v-channel
    # vh = (p%8)*64 + ci*16 + p//8
    ow1r = ow1.reshape(L, 2, P, KH, 3, 3)  # [l, co, jo, vh, ty, tx]
    w1 = np.empty((L, 2, P, 9, 4, P), np.float32)
    j = np.arange(P)
    for ci in range(4):
        vh = (j % 8) * 64 + ci * 16 + j // 8
        sub = ow1r[:, :, :, vh, :, :]          # [l, co, jo, p, ty, tx]
        w1[:, :, :, :, ci, :] = sub.transpose(0, 1, 3, 4, 5, 2).reshape(
            L, 2, P, 9, P)
    d["w1d"] = np.ascontiguousarray(w1.reshape(L, 2, P, 36 * P)).astype(bf)

    # conv2 weights: [L, co, p(cin), tap, ci, jo]
    a2 = ow2.reshape(L, 2, P, 2, P, 3, 3)      # [l, co, jo, ci, p, ty, tx]
    w2 = a2.transpose(0, 1, 4, 5, 6, 3, 2).reshape(L, 2, P, 9, 2, P)
    d["w2d"] = np.ascontiguousarray(w2.reshape(L, 2, P, 18 * P)).astype(bf)

    # per-layer consts: [bng0, bng1, bnb0, bnb1, gob0, gob1, gam, 0]
    cst = np.zeros((L, P, 8), np.float32)
    bn_g, bn_b = inputs["bn_g"], inputs["bn_b"]
    gob2 = gammas[:, None] * ob2
    for co in range(2):
        cst[:, :, co] = bn_g[:, co * P:(co + 1) * P]
        cst[:, :, 2 + co] = bn_b[:, co * P:(co + 1) * P]
        cst[:, :, 4 + co] = gob2[:, co * P:(co + 1) * P]
    cst[:, :, 6] = gammas[:, None]
    d["cstd"] = np.ascontiguousarray(cst)
    return d


def _in_maps(inputs):
    shared = _host_prep(inputs)
    x = np.ascontiguousarray(inputs["x"].reshape(B, C, HW)).astype(np.float32)
    in_maps = []
    for c in range(NC):
        m = dict(shared)
        m["xin"] = x[c]
        in_maps.append(m)
    return in_maps


def kernel(**inputs):
    if "nc" not in _compiled:
        _compiled["nc"] = _build()
    nc = _compiled["nc"]
    in_maps = _in_maps(inputs)
    res = bass_utils.run_bass_kernel_spmd(nc, in_maps, core_ids=list(range(NC)))
    outs = np.stack([res.results[c]["out"] for c in range(NC)])
    return outs.reshape(B, C, H, W).astype(np.float32)
